# revision 5
# baseline (speedup 1.0000x reference)
"""DepthAwareBokehDFN Trainium2 kernel, v3.

Per image: x = concat(rgb, depth) (4ch) -> conv3x3(64)+relu ->
conv3x3(64)+relu -> conv3x3(81) -> softmax over 81 taps -> 9x9 dynamic
filtering of rgb.  Data parallel over 8 cores; shard = (batch, H-half),
R=192 output rows per core.  Halos recomputed from DRAM.

v3 dataflow per core (PE-lean row-streamed convs):
  - conv1: host pair-packs im2col input as ODD pairs (rows 2k-1, 2k);
    one K=72 block-diag matmul per pair -> h1w odd slots, no fills.
  - conv2: dense M=128 packing: pair (w, w+1) shares one PSUM tile
    (row w at parts 0:64, w+1 at 64:128); 6 K=128 matmuls/pair with
    75%-dense lhsT blocks A_kw=[[kh0,0],[kh1,kh0]] (rhs = h1w slot w-1)
    and B_kw=[[kh2,kh1],[0,kh2]] (rhs = slot w+1).  ONE relu evac/pair.
  - conv3: 10 matmuls/pair: 6 vertical (kh0,kh1)-dominoes from h2w
    (K=128, shared lhsT between the rows) + per row one horizontal
    (kh2kw0,kh2kw1)-domino from h2s (slot j = (row j, row j shifted
    left 1)) + one K=64 single (kh2kw2).
  - exp ACT writes estg DIRECTLY with a scattered AP:
    estg[t, 128*l + (r' + 32*qx)]; one XBAR dma_start_transpose per
    quarter-strip -> eb[p, 96*l + t], p = r' + 32*qx.
  - bokeh per strip: DVE does tap-product muls + 40-fold + tap80 add;
    Pool (gpsimd) does 20-fold + 10-fold + reduce; DVE normalizes.
  - engine/queue placement: XBARs exclusively on SP hwdge; all window
    fills on scalar hwdge; x36 (prefetched one 8-pair batch ahead),
    rgb and out DMAs on gpsimd swdge.  PSUM: 4 tags x 2 bufs = 8 banks.
"""

import os
import sys
import numpy as np

if "/opt/trn_rl_repo" not in sys.path:
    sys.path.insert(0, "/opt/trn_rl_repo")

import concourse.bass as bass  # noqa: E402
import concourse.bacc as bacc  # noqa: E402
import concourse.mybir as mybir  # noqa: E402
import concourse.tile as tile  # noqa: E402

F32 = mybir.dt.float32
F16 = mybir.dt.float16

B, H, W = 4, 384, 384
NC_ = 8          # cores
RS = 392         # row slot stride (elements) in window buffers
QW = 96          # quarter-row width
SR = 32          # rows per bokeh strip (=> 128 partitions = 32*4)
NS = 8           # slots in h1w/h2s windows
NS2 = 16         # slots in h2w (wrap-safety vs same-iter fills)
NSX = 16         # slots in x36 window (8-pair batches, double-buffered)
RGBW = 104       # rgb halo block row width (96 + 8)
RGBF = 27 * RGBW  # rgb halo block elems per partition (3ch * 9dy * 104)
EB = 96 * 96     # eb free elems per partition (l-major, 96 tap slots)
ESF = SR * W     # ESTG free elems (12288)

# weight-table column layout (fp16 table)
C_L1 = 0                  # block-diag conv1 lhsT (72,128)
C_A = 128                 # conv2 dense A_kw: 3x (128,128)
C_B = C_A + 384           # conv2 dense B_kw: 3x (128,128)
C_LV = C_B + 384          # conv3 vertical [kh0;kh1]_kw: 3x (128,81)
C_HD = C_LV + 243         # conv3 horizontal [kh2kw0;kh2kw1]: (128,81)
C_HS = C_HD + 81          # conv3 single kh2kw2: (64,81)
WCOLS = C_HS + 81

AF = mybir.ActivationFunctionType
ALU = mybir.AluOpType

DIRECT_EXP = True   # exp ACT writes estg directly (else er + DVE scatter)
POOL_FOLD = True    # 20/10-folds + reduce on gpsimd (else DVE)


def build_core_program(R=192, dbg_tap=None):
    """Per-core Bass program.  R = output rows (multiple of SR)."""
    assert R % SR == 0
    nstrip = R // SR
    P1 = (R + 4) // 2   # conv1 pairs (rows -1 .. R+2)

    nc = bacc.Bacc("TRN2", debug=False, enable_asserts=False,
                   num_devices=NC_, enable_partition_id=False,
                   num_swdge_queues=4)

    x36d = nc.dram_tensor("x36d", [72, P1, RS], F16,
                          kind="ExternalInput").ap()
    rgbs = nc.dram_tensor("rgbs", [nstrip * 128, RGBF], F16,
                          kind="ExternalInput").ap()
    wtsb = nc.dram_tensor("wtsb", [128, WCOLS], F16,
                          kind="ExternalInput").ap()
    wtb = nc.dram_tensor("wtb", [128, 3], F32, kind="ExternalInput").ap()
    out = nc.dram_tensor("out", [3, R, W], F32, kind="ExternalOutput").ap()
    dbg = None
    if dbg_tap == "E":
        dbg = nc.dram_tensor("dbg", [nstrip, 96, ESF], F16,
                             kind="ExternalOutput").ap()
    elif dbg_tap == "eb":
        dbg = nc.dram_tensor("dbg", [nstrip, 128, EB], F16,
                             kind="ExternalOutput").ap()
    elif dbg_tap == "h1":
        dbg = nc.dram_tensor("dbg", [64, R + 4, RS], F16,
                             kind="ExternalOutput").ap()
    elif dbg_tap == "h2":
        dbg = nc.dram_tensor("dbg", [64, R + 4, RS], F16,
                             kind="ExternalOutput").ap()

    def so1(j):  # h1w slot of odd row j (pair (j, j+1)), j = -1, 1, 3, ...
        return ((j + 1) // 2) % NS

    def so2(j):  # h2w slot of row j (holds (row j, row j+1) when even)
        return (j + 2) % NS2

    def sos(j):  # h2s slot of row j: (row j, row j shifted left 1)
        return (j + 2) % NS

    with tile.TileContext(nc) as tc:
        with (
            tc.tile_pool(name="singles", bufs=1) as singles,
            tc.tile_pool(name="ostg_pool", bufs=2) as ostg_pool,
            tc.tile_pool(name="er_pool", bufs=4) as er_pool,
            tc.tile_pool(name="psum", bufs=2, space="PSUM") as psum,
        ):
            # ---- persistent SBUF state ----
            wtsb_sb = singles.tile([128, WCOLS], F16)
            nc.sync.dma_start(out=wtsb_sb, in_=wtsb)
            wtb_sb = singles.tile([128, 3], F32)
            nc.sync.dma_start(out=wtb_sb, in_=wtb)
            h1w = singles.tile([128, NS * RS], F16)
            h2w = singles.tile([128, NS2 * RS], F16)
            h2s = singles.tile([128, NS * RS], F16)
            x36 = singles.tile([72, NSX * RS], F16)
            estg = [singles.tile([96, ESF], F16, name=f"estg{i}")
                    for i in range(2)]
            eb = singles.tile([128, EB], F16)
            rgbb = [singles.tile([128, RGBF], F16, name=f"rgbb{i}")
                    for i in range(2)]
            tmpP = singles.tile([128, 96 * 81], F16)
            sA40 = singles.tile([128, 2, 40 * QW], F16)
            s20 = singles.tile([128, 20 * QW], F16)
            s10 = singles.tile([128, 10 * QW], F16)
            s5 = singles.tile([128, 5 * QW], F16)
            uacc = singles.tile([128, 4, QW], F32)

            nc.vector.memset(h1w, 0.0)
            nc.vector.memset(h2w, 0.0)
            nc.vector.memset(h2s, 0.0)
            nc.vector.memset(x36, 0.0)
            for es_ in estg:
                nc.vector.memset(es_[0:96, :], 0.0)

            # weight slices
            l1 = wtsb_sb[0:72, C_L1:C_L1 + 128]
            wA = [wtsb_sb[0:128, C_A + 128 * k:C_A + 128 * (k + 1)]
                  for k in range(3)]
            wB = [wtsb_sb[0:128, C_B + 128 * k:C_B + 128 * (k + 1)]
                  for k in range(3)]
            lv = [wtsb_sb[0:128, C_LV + 81 * k:C_LV + 81 * (k + 1)]
                  for k in range(3)]
            hd = wtsb_sb[0:128, C_HD:C_HD + 81]
            hs = wtsb_sb[0:64, C_HS:C_HS + 81]

            def bias(col, lo, hi):
                return wtb_sb[lo:hi, col:col + 1]

            # ---------------- emission helpers ----------------
            def emit_x36_batch(k0):
                # prefetch pair-slots k0 .. k0+7 (window slot = k % NSX)
                n = min(8, P1 - k0)
                if n <= 0:
                    return
                F = NSX * RS
                dst = bass.AP(tensor=x36.tensor, offset=(k0 % NSX) * RS,
                              ap=[[F, 72], [RS, n], [1, RS]])
                src = bass.AP(tensor=x36d.tensor, offset=k0 * RS,
                              ap=[[P1 * RS, 72], [RS, n], [1, RS]])
                nc.gpsimd.dma_start(out=dst, in_=src)

            def emit_conv1(k, ps):
                # one K=72 block-diag matmul: psum 0:64 = row 2k-1,
                # 64:128 = row 2k
                rhs = x36[0:72, (k % NSX) * RS + 1:(k % NSX) * RS + 385]
                nc.tensor.matmul(out=ps[0:128, 0:384], lhsT=l1, rhs=rhs,
                                 start=True, stop=True)

            def emit_conv2(w, ps):
                # dense M=128: rows (w, w+1) in one PSUM tile
                sa = so1(w - 1) * RS
                sb = so1(w + 1) * RS
                for kw in range(3):
                    nc.tensor.matmul(out=ps[0:128, 0:384], lhsT=wA[kw],
                                     rhs=h1w[0:128, sa + kw:sa + kw + 384],
                                     start=(kw == 0), stop=False)
                for kw in range(3):
                    nc.tensor.matmul(out=ps[0:128, 0:384], lhsT=wB[kw],
                                     rhs=h1w[0:128, sb + kw:sb + kw + 384],
                                     start=False, stop=(kw == 2))

            def emit_conv2_evac_fills(w, ps):
                sl = so2(w) * RS
                nc.scalar.activation(
                    out=h2w[0:128, sl + 1:sl + 385],
                    in_=ps[0:128, 0:384], func=AF.Relu, bias=bias(1, 0, 128))
                # h2w odd slots: c1[w+1] <- c2[w]; c2[w-1] <- c1[w]
                nc.scalar.dma_start(
                    out=h2w[0:64, so2(w + 1) * RS:(so2(w + 1) + 1) * RS],
                    in_=h2w[64:128, sl:sl + RS])
                nc.scalar.dma_start(
                    out=h2w[64:128, so2(w - 1) * RS:(so2(w - 1) + 1) * RS],
                    in_=h2w[0:64, sl:sl + RS])
                # h2s: c1[j] = row j, c2[j] = row j shifted left 1
                nc.scalar.dma_start(
                    out=h2s[0:64, sos(w) * RS:(sos(w) + 1) * RS],
                    in_=h2w[0:64, sl:sl + RS])
                nc.scalar.dma_start(
                    out=h2s[0:64, sos(w + 1) * RS:(sos(w + 1) + 1) * RS],
                    in_=h2w[64:128, sl:sl + RS])
                nc.scalar.dma_start(
                    out=h2s[64:128, sos(w) * RS:sos(w) * RS + RS - 1],
                    in_=h2w[0:64, sl + 1:sl + RS])
                nc.scalar.dma_start(
                    out=h2s[64:128, sos(w + 1) * RS:sos(w + 1) * RS + RS - 1],
                    in_=h2w[64:128, sl + 1:sl + RS])

            def emit_conv3(v, pse, pso):
                # 6 vertical dominoes (shared lhsT per kw) + 2 horizontal
                # dominoes + 2 kh2kw2 singles
                sa = so2(v - 1) * RS
                sb = so2(v) * RS
                for kw in range(3):
                    nc.tensor.matmul(out=pse[0:81, 0:384], lhsT=lv[kw],
                                     rhs=h2w[0:128, sa + kw:sa + kw + 384],
                                     start=(kw == 0), stop=False)
                    nc.tensor.matmul(out=pso[0:81, 0:384], lhsT=lv[kw],
                                     rhs=h2w[0:128, sb + kw:sb + kw + 384],
                                     start=(kw == 0), stop=False)
                ha = sos(v + 1) * RS
                hb = sos(v + 2) * RS
                nc.tensor.matmul(out=pse[0:81, 0:384], lhsT=hd,
                                 rhs=h2s[0:128, ha:ha + 384],
                                 start=False, stop=False)
                nc.tensor.matmul(out=pso[0:81, 0:384], lhsT=hd,
                                 rhs=h2s[0:128, hb:hb + 384],
                                 start=False, stop=False)
                nc.tensor.matmul(out=pse[0:81, 0:384], lhsT=hs,
                                 rhs=h2s[0:64, ha + 2:ha + 386],
                                 start=False, stop=True)
                nc.tensor.matmul(out=pso[0:81, 0:384], lhsT=hs,
                                 rhs=h2s[0:64, hb + 2:hb + 386],
                                 start=False, stop=True)

            def emit_exp(v, i, ps):
                # exp(logits+b3) of row v+i
                es = estg[(v // SR) % 2]
                r = v % SR + i
                if DIRECT_EXP:
                    # direct scattered write: estg[t, 128*l + r + 32*qx]
                    dst = bass.AP(tensor=es.tensor, offset=r,
                                  ap=[[ESF, 81], [32, 4], [128, 96]])
                    src = bass.AP(tensor=ps.tensor, offset=0,
                                  ap=[[384, 81], [96, 4], [1, 96]])
                    nc.scalar.activation(out=dst, in_=src, func=AF.Exp,
                                         bias=bias(2, 0, 81))
                    return None
                return ps  # caller stages via er

            def emit_exp_er(v, er, i, ps):
                nc.scalar.activation(out=er[0:81, 384 * i:384 * (i + 1)],
                                     in_=ps[0:81, 0:384],
                                     func=AF.Exp, bias=bias(2, 0, 81))

            def emit_scatter(v, er):
                # fallback: DVE scatter into estg[t, 128*l + r + 32*qx]
                es = estg[(v // SR) % 2]
                r = v % SR
                dst = bass.AP(tensor=es.tensor, offset=r,
                              ap=[[ESF, 81], [1, 2], [32, 4], [128, 96]])
                src = bass.AP(tensor=er.tensor, offset=0,
                              ap=[[768, 81], [384, 2], [96, 4], [1, 96]])
                nc.vector.tensor_copy(out=dst, in_=src)

            def emit_xbar_q(s, q):
                # quarter-strip transpose: eb[p, 96*l + t] for l in 24q..
                es = estg[s % 2]
                dst = bass.AP(tensor=eb.tensor, offset=q * 24 * 96,
                              ap=[[EB, 128], [96, 24], [1, 96]])
                nc.sync.dma_start_transpose(
                    out=dst, in_=es[0:96, q * 3072:(q + 1) * 3072])
                if dbg_tap == "E" and q == 0:
                    nc.gpsimd.dma_start(out=dbg[s], in_=es[0:96, :])

            def emit_rgb_dma(s):
                nc.gpsimd.dma_start(
                    out=rgbb[s % 2][0:128, :],
                    in_=rgbs[s * 128:(s + 1) * 128, :])

            def emit_bokeh(s):
                if dbg_tap == "eb":
                    nc.gpsimd.dma_start(out=dbg[s], in_=eb[0:128, :])
                ostg = ostg_pool.tile([128, 3, QW], F32, name=f"ostg{s}",
                                      tag="ostg")
                fold_eng = nc.gpsimd if POOL_FOLD else nc.vector
                with nc.allow_low_precision("fp16 bokeh by design"):
                    for ch in range(4):
                        par = ch % 2
                        if ch < 3:
                            # tmpP[(l,dy,dx)] = E * rgb_shift  (tap-minor)
                            dst = bass.AP(
                                tensor=tmpP.tensor, offset=0,
                                ap=[[96 * 81, 128], [81, 96], [9, 9],
                                    [1, 9]])
                            ein = bass.AP(
                                tensor=eb.tensor, offset=0,
                                ap=[[EB, 128], [96, 96], [9, 9], [1, 9]])
                            rin = bass.AP(
                                tensor=rgbb[s % 2].tensor,
                                offset=ch * 9 * RGBW,
                                ap=[[RGBF, 128], [1, 96], [RGBW, 9],
                                    [1, 9]])
                            nc.vector.tensor_mul(dst, ein, rin)
                            src_t, tst = tmpP, 81
                        else:
                            src_t, tst = eb, 96
                        srcf = 96 * tst

                        def sap(off, cnt):
                            return bass.AP(tensor=src_t.tensor,
                                           offset=off,
                                           ap=[[srcf, 128], [tst, 96],
                                               [1, cnt]])

                        def a40(cnt, t0=0):
                            return bass.AP(tensor=sA40.tensor,
                                           offset=par * 40 * QW + t0,
                                           ap=[[2 * 40 * QW, 128],
                                               [40, 96], [1, cnt]])

                        # DVE: 80->40 fold, then add tap 80 into column 0
                        nc.vector.tensor_add(a40(40), sap(0, 40),
                                             sap(40, 40))
                        nc.vector.tensor_add(
                            bass.AP(tensor=sA40.tensor,
                                    offset=par * 40 * QW,
                                    ap=[[2 * 40 * QW, 128], [40, 96]]),
                            bass.AP(tensor=sA40.tensor,
                                    offset=par * 40 * QW,
                                    ap=[[2 * 40 * QW, 128], [40, 96]]),
                            bass.AP(tensor=src_t.tensor, offset=80,
                                    ap=[[srcf, 128], [tst, 96]]))

                        # Pool: 40->20->10 folds + reduce
                        def s20ap(cnt, t0=0):
                            return bass.AP(tensor=s20.tensor, offset=t0,
                                           ap=[[20 * QW, 128], [20, 96],
                                               [1, cnt]])

                        def s10ap(cnt, t0=0):
                            return bass.AP(tensor=s10.tensor, offset=t0,
                                           ap=[[10 * QW, 128], [10, 96],
                                               [1, cnt]])

                        def s5ap(cnt, t0=0):
                            return bass.AP(tensor=s5.tensor, offset=t0,
                                           ap=[[5 * QW, 128], [5, 96],
                                               [1, cnt]])

                        fold_eng.tensor_add(s20ap(20), a40(20),
                                            a40(20, t0=20))
                        fold_eng.tensor_add(s10ap(10), s20ap(10),
                                            s20ap(10, t0=10))
                        fold_eng.tensor_add(s5ap(5), s10ap(5),
                                            s10ap(5, t0=5))
                        nc.vector.tensor_reduce(
                            out=uacc[0:128, ch, :], in_=s5ap(5),
                            axis=mybir.AxisListType.X, op=ALU.add)

                    nc.vector.reciprocal(uacc[0:128, 3, :],
                                         uacc[0:128, 3, :])
                    for ch in range(3):
                        nc.vector.tensor_mul(ostg[0:128, ch, :],
                                             uacc[0:128, ch, :],
                                             uacc[0:128, 3, :])

                for ch in range(3):
                    dst = bass.AP(tensor=out.tensor,
                                  offset=ch * R * W + s * SR * W,
                                  ap=[[QW, 4], [W, SR], [1, QW]])
                    src = bass.AP(tensor=ostg.tensor, offset=ch * QW,
                                  ap=[[3 * QW, 128], [1, QW]])
                    nc.gpsimd.dma_start(out=dst, in_=src)

            # ---------------- main row loop ----------------
            emit_rgb_dma(0)
            emit_x36_batch(0)
            kmax = R // 2 + 6
            for k in range(kmax + 1):
                j1 = 2 * k - 1          # conv1 pair (j1, j1+1)
                if j1 <= R + 1:
                    if k % 8 == 0:
                        emit_x36_batch(k + 8)
                    ps1 = psum.tile([128, 384], F32, tag="c1",
                                    name=f"c1_{k}")
                    emit_conv1(k, ps1)
                    nc.scalar.activation(
                        out=h1w[0:128,
                                so1(j1) * RS + 1:so1(j1) * RS + 385],
                        in_=ps1[0:128, 0:384], func=AF.Relu,
                        bias=bias(0, 0, 128))
                    if dbg_tap == "h1":
                        sl = so1(j1) * RS
                        nc.gpsimd.dma_start(
                            out=dbg[:, j1 + 1, :],
                            in_=h1w[0:64, sl:sl + RS])
                        nc.gpsimd.dma_start(
                            out=dbg[:, j1 + 2, :],
                            in_=h1w[64:128, sl:sl + RS])

                w = 2 * k - 8           # conv2 pair (w, w+1), w even
                if 0 <= w <= R:
                    ps2 = psum.tile([128, 384], F32, tag="c2",
                                    name=f"c2_{k}")
                    emit_conv2(w, ps2)
                    emit_conv2_evac_fills(w, ps2)
                    if dbg_tap == "h2":
                        sl = so2(w) * RS
                        nc.gpsimd.dma_start(
                            out=dbg[:, w + 1, :],
                            in_=h2w[0:64, sl:sl + RS])
                        nc.gpsimd.dma_start(
                            out=dbg[:, w + 2, :],
                            in_=h2w[64:128, sl:sl + RS])

                v = 2 * k - 14          # conv3 pair (v, v+1), v even
                if 0 <= v <= R - 2:
                    pse = psum.tile([128, 384], F32, tag="c3e",
                                    name=f"c3e_{k}")
                    pso = psum.tile([128, 384], F32, tag="c3o",
                                    name=f"c3o_{k}")
                    emit_conv3(v, pse, pso)
                    if DIRECT_EXP:
                        emit_exp(v, 0, pse)
                        emit_exp(v, 1, pso)
                    else:
                        er = er_pool.tile([81, 768], F16,
                                          name=f"er_{v}", tag="er")
                        emit_exp_er(v, er, 0, pse)
                        emit_exp_er(v, er, 1, pso)
                        emit_scatter(v, er)
                    if (v + 1) % SR == SR - 1:
                        s = v // SR
                        for q in range(4):
                            emit_xbar_q(s, q)
                        if s + 1 < nstrip:
                            emit_rgb_dma(s + 1)
                        emit_bokeh(s)

    nc.compile()
    return nc


# ------------------------- host side -------------------------

def prep_weights(w1, b1, w2, b2, w3, b3, flip=False):
    if flip:
        perm = np.array([(8 - t // 9) * 9 + t % 9 for t in range(81)])
        w1 = w1[:, :, ::-1, :]
        w2 = w2[:, :, ::-1, :]
        w3 = w3[perm][:, :, ::-1, :]
        b3 = b3[perm]
    wtsb = np.zeros((128, WCOLS), np.float32)
    l1 = w1.transpose(2, 3, 1, 0).reshape(36, 64)
    wtsb[0:36, C_L1:C_L1 + 64] = l1
    wtsb[36:72, C_L1 + 64:C_L1 + 128] = l1
    for kw in range(3):
        kh0 = w2[:, :, 0, kw].T
        kh1 = w2[:, :, 1, kw].T
        kh2 = w2[:, :, 2, kw].T
        a0 = C_A + 128 * kw
        wtsb[0:64, a0:a0 + 64] = kh0
        wtsb[64:128, a0:a0 + 64] = kh1
        wtsb[64:128, a0 + 64:a0 + 128] = kh0
        b0 = C_B + 128 * kw
        wtsb[0:64, b0:b0 + 64] = kh2
        wtsb[0:64, b0 + 64:b0 + 128] = kh1
        wtsb[64:128, b0 + 64:b0 + 128] = kh2
        v0 = C_LV + 81 * kw
        wtsb[0:64, v0:v0 + 81] = w3[:, :, 0, kw].T
        wtsb[64:128, v0:v0 + 81] = w3[:, :, 1, kw].T
    wtsb[0:64, C_HD:C_HD + 81] = w3[:, :, 2, 0].T
    wtsb[64:128, C_HD:C_HD + 81] = w3[:, :, 2, 1].T
    wtsb[0:64, C_HS:C_HS + 81] = w3[:, :, 2, 2].T
    wtb = np.zeros((128, 3), np.float32)
    wtb[0:64, 0] = b1
    wtb[64:128, 0] = b1
    wtb[0:64, 1] = b2
    wtb[64:128, 1] = b2
    wtb[0:81, 2] = b3
    return wtsb.astype(np.float16), wtb


def prep_shard(x, rgb_b, R):
    """x: (4,H,W) fp32 of one (possibly flipped) image; rgb_b: (3,H,W).
    Shard = rows 0..R-1 out; top edge is the image edge (zero pad),
    bottom halo rows come from the rest of the image.

    Returns (x36d, rgbs): odd-pair-packed im2col'd conv1 input and
    per-strip rgb halo blocks with partition p = r' + 32*qx."""
    # conv1 rows -1 .. R+2 (nrows = R+4); taps read x rows -2 .. R+3
    nrows = R + 4
    xp = np.zeros((4, R + 8, RS), np.float32)  # image row i at xp[i+3]
    hi = min(R + 4, H)
    xp[:, 3:3 + hi, 1:385] = x[:, 0:hi, :]
    x36f = np.zeros((36, nrows, RS), np.float16)
    for kh in range(3):
        # conv1 row index i (row r = i-1): tap row r+kh-1 -> xp[i+kh+1]
        sl = xp[:, kh + 1:kh + 1 + nrows, :]
        for kw in range(3):
            blk = np.zeros((4, nrows, RS), np.float32)
            if kw == 0:
                blk[:, :, 1:] = sl[:, :, :-1]
            elif kw == 1:
                blk[:, :, :] = sl
            else:
                blk[:, :, :-1] = sl[:, :, 1:]
            for c in range(4):
                x36f[kh * 12 + kw * 4 + c] = blk[c].astype(np.float16)
    # conv1-out row -1 must be exactly zero (image-edge h1 padding)
    x36f[:, 0, :] = 0
    x36d = np.zeros((72, nrows // 2, RS), np.float16)
    x36d[0:36] = x36f[:, 0::2, :]
    x36d[36:72] = x36f[:, 1::2, :]

    # rgb halo rows -4 .. R+4; partition p = r' + 32*qx
    rgbp = np.zeros((3, R + 8, W + 8), np.float32)
    hi2 = min(R + 4, H)
    rgbp[:, 4:4 + hi2, 4:4 + W] = rgb_b[:, 0:hi2, :]
    nstrip = R // SR
    arr = np.zeros((nstrip * 128, RGBF), np.float16)
    rows = rgbp.astype(np.float16)  # (3, R+8, 392)
    for s in range(nstrip):
        for dy in range(9):
            seg = rows[:, s * SR + dy:s * SR + dy + SR, :]  # (3,SR,392)
            for qx in range(4):
                qseg = seg[:, :, qx * 96:qx * 96 + RGBW]  # (3,SR,104)
                view = arr[s * 128 + 32 * qx:s * 128 + 32 * qx + 32]
                for c in range(3):
                    view[:, (c * 9 + dy) * RGBW:
                         (c * 9 + dy + 1) * RGBW] = qseg[c]
    return x36d, arr


def _prep_inputs(rgb, depth, w1, b1, w2, b2, w3, b3):
    R = H // 2
    x = np.concatenate([rgb, depth], axis=1)  # (B,4,H,W)
    wt_n = prep_weights(w1, b1, w2, b2, w3, b3, flip=False)
    wt_f = prep_weights(w1, b1, w2, b2, w3, b3, flip=True)
    in_maps = []
    for core in range(NC_):
        bi, half = divmod(core, 2)
        if half == 0:
            xi, ri = x[bi], rgb[bi]
            wtsb, wtb = wt_n
        else:
            xi, ri = x[bi, :, ::-1, :], rgb[bi, :, ::-1, :]
            wtsb, wtb = wt_f
        x36d, rgbs = prep_shard(xi, ri, R)
        in_maps.append({"x36d": x36d, "rgbs": rgbs,
                        "wtsb": wtsb, "wtb": wtb})
    return in_maps


_CACHE = {}


def _get_program(R=H // 2, dbg_tap=None):
    key = (R, dbg_tap)
    if key not in _CACHE:
        _CACHE[key] = build_core_program(R, dbg_tap)
    return _CACHE[key]


def kernel(rgb, depth, w1, b1, w2, b2, w3, b3):
    from concourse.bass_utils import run_bass_kernel_spmd
    rgb = np.asarray(rgb, np.float32)
    depth = np.asarray(depth, np.float32)
    nc = _get_program()
    in_maps = _prep_inputs(rgb, depth, np.asarray(w1, np.float32),
                           np.asarray(b1, np.float32),
                           np.asarray(w2, np.float32),
                           np.asarray(b2, np.float32),
                           np.asarray(w3, np.float32),
                           np.asarray(b3, np.float32))
    res = run_bass_kernel_spmd(nc, in_maps, core_ids=list(range(NC_)),
                               trace=bool(int(os.environ.get("KT_TRACE",
                                                             "0"))))
    R = H // 2
    outp = np.zeros((B, 3, H, W), np.float32)
    for core in range(NC_):
        bi, half = divmod(core, 2)
        o = res.results[core]["out"]
        if half == 0:
            outp[bi, :, 0:R, :] = o
        else:
            outp[bi, :, R:H, :] = o[:, ::-1, :]
    kernel.last_result = res
    return outp


if __name__ == "__main__":
    nc = build_core_program(R=int(sys.argv[1]) if len(sys.argv) > 1 else 32)
    print("built ok")


# revision 6
# speedup vs baseline: 1.6458x; 1.6458x over previous
"""DepthAwareBokehDFN Trainium2 kernel, v3.

Per image: x = concat(rgb, depth) (4ch) -> conv3x3(64)+relu ->
conv3x3(64)+relu -> conv3x3(81) -> softmax over 81 taps -> 9x9 dynamic
filtering of rgb.  Data parallel over 8 cores; shard = (batch, H-half),
R=192 output rows per core.  Halos recomputed from DRAM.

v3 dataflow per core (PE-lean row-streamed convs):
  - conv1: host pair-packs im2col input as ODD pairs (rows 2k-1, 2k);
    one K=72 block-diag matmul per pair -> h1w odd slots, no fills.
  - conv2: dense M=128 packing: pair (w, w+1) shares one PSUM tile
    (row w at parts 0:64, w+1 at 64:128); 6 K=128 matmuls/pair with
    75%-dense lhsT blocks A_kw=[[kh0,0],[kh1,kh0]] (rhs = h1w slot w-1)
    and B_kw=[[kh2,kh1],[0,kh2]] (rhs = slot w+1).  ONE relu evac/pair.
  - conv3: 12 matmuls/pair, ALL rhs from native even h2w slots (ZERO
    window-fill DMAs): per pair, rows (v, v+1) share rhs slot v for 6
    K=128 dominoes (lvA=[kh1;kh2] for row v, lvB=[kh0;kh1] for row
    v+1) + 3 K=64 kh0-singles (slot v-2 hi) + 3 K=64 kh2-singles
    (slot v+2 lo).
  - exp ACT -> er (contiguous), DVE scatter -> estg[t, 128*l + p],
    p = 4*r' + qx; one XBAR dma_start_transpose per quarter-strip ->
    eb[p, 96*l + t].
  - bokeh per strip on DVE: tap-product muls, 40/20/10-folds + tap80
    add, reduce, reciprocal normalize.
  - engine/queue placement: XBARs exclusively on SP hwdge; x36
    (prefetched one 8-pair batch ahead), rgb and out DMAs on gpsimd
    swdge.  PSUM: 4 tags x 2 bufs = 8 banks.
"""

import os
import sys
import numpy as np

if "/opt/trn_rl_repo" not in sys.path:
    sys.path.insert(0, "/opt/trn_rl_repo")

import concourse.bass as bass  # noqa: E402
import concourse.bacc as bacc  # noqa: E402
import concourse.mybir as mybir  # noqa: E402
import concourse.tile as tile  # noqa: E402

F32 = mybir.dt.float32
F16 = mybir.dt.float16

B, H, W = 4, 384, 384
NC_ = 8          # cores
RS = 392         # row slot stride (elements) in window buffers
QW = 96          # quarter-row width
SR = 32          # rows per bokeh strip (=> 128 partitions = 32*4)
NS = 8           # slots in h1w window (odd pairs)
NS2 = 8          # slots in h2w window (even pairs)
NSX = 16         # slots in x36 window (8-pair batches, double-buffered)
RGBW = 104       # rgb halo block row width (96 + 8)
RGBF = 27 * RGBW  # rgb halo block elems per partition (3ch * 9dy * 104)
EB = 96 * 96     # eb free elems per partition (l-major, 96 tap slots)
ESF = SR * W     # ESTG free elems (12288)

# weight-table column layout (fp16 table)
C_L1 = 0                  # block-diag conv1 lhsT (72,128)
C_A = 128                 # conv2 dense A_kw: 3x (128,128)
C_B = C_A + 384           # conv2 dense B_kw: 3x (128,128)
C_LVA = C_B + 384         # conv3 row-v dominoes [kh1;kh2]_kw: 3x (128,81)
C_LVB = C_LVA + 243       # conv3 row-v+1 dominoes [kh0;kh1]_kw: 3x (128,81)
C_S = C_LVB + 243         # conv3 singles [kh2 lo; kh0 hi]_kw: 3x (128,81)
WCOLS = C_S + 243

AF = mybir.ActivationFunctionType
ALU = mybir.AluOpType

DIRECT_EXP = False  # exp ACT writes estg directly (else er + DVE scatter)


def build_core_program(R=192, dbg_tap=None):
    """Per-core Bass program.  R = output rows (multiple of SR)."""
    assert R % SR == 0
    nstrip = R // SR
    P1 = (R + 4) // 2   # conv1 pairs (rows -1 .. R+2)

    nc = bacc.Bacc("TRN2", debug=False, enable_asserts=False,
                   num_devices=NC_, enable_partition_id=False,
                   num_swdge_queues=4)

    x36d = nc.dram_tensor("x36d", [72, P1, RS], F16,
                          kind="ExternalInput").ap()
    rgbs = nc.dram_tensor("rgbs", [nstrip * 128, RGBF], F16,
                          kind="ExternalInput").ap()
    wtsb = nc.dram_tensor("wtsb", [128, WCOLS], F16,
                          kind="ExternalInput").ap()
    wtb = nc.dram_tensor("wtb", [128, 3], F32, kind="ExternalInput").ap()
    out = nc.dram_tensor("out", [3, R, W], F32, kind="ExternalOutput").ap()
    dbg = None
    if dbg_tap == "E":
        dbg = nc.dram_tensor("dbg", [nstrip, 96, ESF], F16,
                             kind="ExternalOutput").ap()
    elif dbg_tap == "eb":
        dbg = nc.dram_tensor("dbg", [nstrip, 128, EB], F16,
                             kind="ExternalOutput").ap()
    elif dbg_tap == "h1":
        dbg = nc.dram_tensor("dbg", [64, R + 4, RS], F16,
                             kind="ExternalOutput").ap()
    elif dbg_tap == "h2":
        dbg = nc.dram_tensor("dbg", [64, R + 4, RS], F16,
                             kind="ExternalOutput").ap()

    def so1(j):  # h1w slot of odd row j (pair (j, j+1)), j = -1, 1, 3, ...
        return ((j + 1) // 2) % NS

    def se2(w):  # h2w slot of even conv2 pair (w, w+1), w = -2, 0, 2, ...
        return (w // 2 + 1) % NS2

    with tile.TileContext(nc) as tc:
        with (
            tc.tile_pool(name="singles", bufs=1) as singles,
            tc.tile_pool(name="ostg_pool", bufs=2) as ostg_pool,
            tc.tile_pool(name="er_pool", bufs=4) as er_pool,
            tc.tile_pool(name="psum", bufs=2, space="PSUM") as psum,
        ):
            # ---- persistent SBUF state ----
            wtsb_sb = singles.tile([128, WCOLS], F16)
            nc.sync.dma_start(out=wtsb_sb, in_=wtsb)
            wtb_sb = singles.tile([128, 3], F32)
            nc.sync.dma_start(out=wtb_sb, in_=wtb)
            h1w = singles.tile([128, NS * RS], F16)
            h2w = singles.tile([128, NS2 * RS], F16)
            x36 = singles.tile([72, NSX * RS], F16)
            estg = [singles.tile([96, ESF], F16, name=f"estg{i}")
                    for i in range(2)]
            eb = singles.tile([128, EB], F16)
            rgbb = [singles.tile([128, RGBF], F16, name=f"rgbb{i}")
                    for i in range(2)]
            tmpP = singles.tile([128, 96 * 81], F16)
            sA40 = singles.tile([128, 40 * QW], F16)
            s20 = singles.tile([128, 20 * QW], F16)
            s10 = singles.tile([128, 10 * QW], F16)
            uacc = singles.tile([128, 4, QW], F32)

            nc.vector.memset(h1w, 0.0)
            nc.vector.memset(h2w, 0.0)
            nc.vector.memset(x36, 0.0)
            for es_ in estg:
                nc.vector.memset(es_[0:96, :], 0.0)

            # weight slices
            l1 = wtsb_sb[0:72, C_L1:C_L1 + 128]
            wA = [wtsb_sb[0:128, C_A + 128 * k:C_A + 128 * (k + 1)]
                  for k in range(3)]
            wB = [wtsb_sb[0:128, C_B + 128 * k:C_B + 128 * (k + 1)]
                  for k in range(3)]
            lvA = [wtsb_sb[0:128, C_LVA + 81 * k:C_LVA + 81 * (k + 1)]
                   for k in range(3)]
            lvB = [wtsb_sb[0:128, C_LVB + 81 * k:C_LVB + 81 * (k + 1)]
                   for k in range(3)]
            sg2 = [wtsb_sb[0:64, C_S + 81 * k:C_S + 81 * (k + 1)]
                   for k in range(3)]
            sg0 = [wtsb_sb[64:128, C_S + 81 * k:C_S + 81 * (k + 1)]
                   for k in range(3)]

            def bias(col, lo, hi):
                return wtb_sb[lo:hi, col:col + 1]

            # ---------------- emission helpers ----------------
            def emit_x36_batch(k0):
                # prefetch pair-slots k0 .. k0+7 (window slot = k % NSX)
                n = min(8, P1 - k0)
                if n <= 0:
                    return
                F = NSX * RS
                dst = bass.AP(tensor=x36.tensor, offset=(k0 % NSX) * RS,
                              ap=[[F, 72], [RS, n], [1, RS]])
                src = bass.AP(tensor=x36d.tensor, offset=k0 * RS,
                              ap=[[P1 * RS, 72], [RS, n], [1, RS]])
                nc.gpsimd.dma_start(out=dst, in_=src)

            def emit_conv1(k, ps):
                # one K=72 block-diag matmul: psum 0:64 = row 2k-1,
                # 64:128 = row 2k
                rhs = x36[0:72, (k % NSX) * RS + 1:(k % NSX) * RS + 385]
                nc.tensor.matmul(out=ps[0:128, 0:384], lhsT=l1, rhs=rhs,
                                 start=True, stop=True)

            def emit_conv2(w, ps):
                # dense M=128: rows (w, w+1) in one PSUM tile
                sa = so1(w - 1) * RS
                sb = so1(w + 1) * RS
                for kw in range(3):
                    nc.tensor.matmul(out=ps[0:128, 0:384], lhsT=wA[kw],
                                     rhs=h1w[0:128, sa + kw:sa + kw + 384],
                                     start=(kw == 0), stop=False)
                for kw in range(3):
                    nc.tensor.matmul(out=ps[0:128, 0:384], lhsT=wB[kw],
                                     rhs=h1w[0:128, sb + kw:sb + kw + 384],
                                     start=False, stop=(kw == 2))

            def emit_conv2_evac(w, ps):
                sl = se2(w) * RS
                nc.scalar.activation(
                    out=h2w[0:128, sl + 1:sl + 385],
                    in_=ps[0:128, 0:384], func=AF.Relu, bias=bias(1, 0, 128))

            def emit_conv3(v, pse, pso):
                # 6 K=128 dominoes sharing rhs slot v (lvA: row v gets
                # kh1,kh2; lvB: row v+1 gets kh0,kh1) + 3 K=64
                # kh0-singles (slot v-2 hi = row v-1) + 3 K=64
                # kh2-singles (slot v+2 lo = row v+2)
                sv = se2(v) * RS
                sm = se2(v - 2) * RS
                sp = se2(v + 2) * RS
                for kw in range(3):
                    rhs = h2w[0:128, sv + kw:sv + kw + 384]
                    nc.tensor.matmul(out=pse[0:81, 0:384], lhsT=lvA[kw],
                                     rhs=rhs, start=(kw == 0), stop=False)
                    nc.tensor.matmul(out=pso[0:81, 0:384], lhsT=lvB[kw],
                                     rhs=rhs, start=(kw == 0), stop=False)
                for kw in range(3):
                    nc.tensor.matmul(out=pse[0:81, 0:384], lhsT=sg0[kw],
                                     rhs=h2w[64:128, sm + kw:sm + kw + 384],
                                     start=False, stop=(kw == 2))
                    nc.tensor.matmul(out=pso[0:81, 0:384], lhsT=sg2[kw],
                                     rhs=h2w[0:64, sp + kw:sp + kw + 384],
                                     start=False, stop=(kw == 2))

            def emit_exp(v, i, ps):
                # exp(logits+b3) of row v+i
                es = estg[(v // SR) % 2]
                r = v % SR + i
                if DIRECT_EXP:
                    # direct scattered write: estg[t, 128*l + 4*r + qx]
                    dst = bass.AP(tensor=es.tensor, offset=4 * r,
                                  ap=[[ESF, 81], [128, 96], [1, 4]])
                    src = bass.AP(tensor=ps.tensor, offset=0,
                                  ap=[[384, 81], [1, 96], [96, 4]])
                    nc.scalar.activation(out=dst, in_=src, func=AF.Exp,
                                         bias=bias(2, 0, 81))
                    return None
                return ps  # caller stages via er

            def emit_exp_er(v, er, i, ps):
                nc.scalar.activation(out=er[0:81, 384 * i:384 * (i + 1)],
                                     in_=ps[0:81, 0:384],
                                     func=AF.Exp, bias=bias(2, 0, 81))

            def emit_scatter(v, er):
                # DVE scatter into estg[t, 128*l + 4*r + qx]; (i, l, qx)
                # iteration keeps dst runs 4-elem contiguous
                es = estg[(v // SR) % 2]
                r = v % SR
                dst = bass.AP(tensor=es.tensor, offset=4 * r,
                              ap=[[ESF, 81], [4, 2], [128, 96], [1, 4]])
                src = bass.AP(tensor=er.tensor, offset=0,
                              ap=[[768, 81], [384, 2], [1, 96], [96, 4]])
                nc.vector.tensor_copy(out=dst, in_=src)

            def emit_xbar_q(s, q):
                # quarter-strip transpose: eb[p, 96*l + t] for l in 24q..
                es = estg[s % 2]
                dst = bass.AP(tensor=eb.tensor, offset=q * 24 * 96,
                              ap=[[EB, 128], [96, 24], [1, 96]])
                nc.sync.dma_start_transpose(
                    out=dst, in_=es[0:96, q * 3072:(q + 1) * 3072])
                if dbg_tap == "E" and q == 0:
                    nc.gpsimd.dma_start(out=dbg[s], in_=es[0:96, :])

            def emit_rgb_dma(s):
                nc.gpsimd.dma_start(
                    out=rgbb[s % 2][0:128, :],
                    in_=rgbs[s * 128:(s + 1) * 128, :])

            def emit_bokeh(s):
                if dbg_tap == "eb":
                    nc.gpsimd.dma_start(out=dbg[s], in_=eb[0:128, :])
                ostg = ostg_pool.tile([128, 3, QW], F32, name=f"ostg{s}",
                                      tag="ostg")
                with nc.allow_low_precision("fp16 bokeh by design"):
                    for ch in range(4):
                        if ch < 3:
                            # tmpP[(l,dy,dx)] = E * rgb_shift  (tap-minor)
                            dst = bass.AP(
                                tensor=tmpP.tensor, offset=0,
                                ap=[[96 * 81, 128], [81, 96], [9, 9],
                                    [1, 9]])
                            ein = bass.AP(
                                tensor=eb.tensor, offset=0,
                                ap=[[EB, 128], [96, 96], [9, 9], [1, 9]])
                            rin = bass.AP(
                                tensor=rgbb[s % 2].tensor,
                                offset=ch * 9 * RGBW,
                                ap=[[RGBF, 128], [1, 96], [RGBW, 9],
                                    [1, 9]])
                            nc.vector.tensor_mul(dst, ein, rin)
                            src_t, tst = tmpP, 81
                        else:
                            src_t, tst = eb, 96
                        srcf = 96 * tst

                        def sap(off, cnt):
                            return bass.AP(tensor=src_t.tensor,
                                           offset=off,
                                           ap=[[srcf, 128], [tst, 96],
                                               [1, cnt]])

                        def a40(cnt, t0=0):
                            return bass.AP(tensor=sA40.tensor,
                                           offset=t0,
                                           ap=[[40 * QW, 128],
                                               [40, 96], [1, cnt]])

                        def s20ap(cnt, t0=0):
                            return bass.AP(tensor=s20.tensor, offset=t0,
                                           ap=[[20 * QW, 128], [20, 96],
                                               [1, cnt]])

                        def s10ap(cnt, t0=0):
                            return bass.AP(tensor=s10.tensor, offset=t0,
                                           ap=[[10 * QW, 128], [10, 96],
                                               [1, cnt]])

                        # 80->40 fold, then add tap 80 into column 0
                        nc.vector.tensor_add(a40(40), sap(0, 40),
                                             sap(40, 40))
                        nc.vector.tensor_add(
                            bass.AP(tensor=sA40.tensor, offset=0,
                                    ap=[[40 * QW, 128], [40, 96]]),
                            bass.AP(tensor=sA40.tensor, offset=0,
                                    ap=[[40 * QW, 128], [40, 96]]),
                            bass.AP(tensor=src_t.tensor, offset=80,
                                    ap=[[srcf, 128], [tst, 96]]))
                        nc.vector.tensor_add(s20ap(20), a40(20),
                                             a40(20, t0=20))
                        nc.vector.tensor_add(s10ap(10), s20ap(10),
                                             s20ap(10, t0=10))
                        nc.vector.tensor_reduce(
                            out=uacc[0:128, ch, :], in_=s10ap(10),
                            axis=mybir.AxisListType.X, op=ALU.add)

                    nc.vector.reciprocal(uacc[0:128, 3, :],
                                         uacc[0:128, 3, :])
                    for ch in range(3):
                        nc.vector.tensor_mul(ostg[0:128, ch, :],
                                             uacc[0:128, ch, :],
                                             uacc[0:128, 3, :])

                for ch in range(3):
                    dst = bass.AP(tensor=out.tensor,
                                  offset=ch * R * W + s * SR * W,
                                  ap=[[W, SR], [QW, 4], [1, QW]])
                    src = bass.AP(tensor=ostg.tensor, offset=ch * QW,
                                  ap=[[3 * QW, 128], [1, QW]])
                    nc.gpsimd.dma_start(out=dst, in_=src)

            # ---------------- main row loop ----------------
            emit_rgb_dma(0)
            emit_x36_batch(0)
            kmax = R // 2 + 6
            for k in range(kmax + 1):
                j1 = 2 * k - 1          # conv1 pair (j1, j1+1)
                if j1 <= R + 1:
                    if k % 8 == 0:
                        emit_x36_batch(k + 8)
                    ps1 = psum.tile([128, 384], F32, tag="c1",
                                    name=f"c1_{k}")
                    emit_conv1(k, ps1)
                    nc.scalar.activation(
                        out=h1w[0:128,
                                so1(j1) * RS + 1:so1(j1) * RS + 385],
                        in_=ps1[0:128, 0:384], func=AF.Relu,
                        bias=bias(0, 0, 128))
                    if dbg_tap == "h1":
                        sl = so1(j1) * RS
                        nc.gpsimd.dma_start(
                            out=dbg[:, j1 + 1, :],
                            in_=h1w[0:64, sl:sl + RS])
                        nc.gpsimd.dma_start(
                            out=dbg[:, j1 + 2, :],
                            in_=h1w[64:128, sl:sl + RS])

                w = 2 * k - 8           # conv2 pair (w, w+1), w even
                if 0 <= w <= R:
                    ps2 = psum.tile([128, 384], F32, tag="c2",
                                    name=f"c2_{k}")
                    emit_conv2(w, ps2)
                    emit_conv2_evac(w, ps2)
                    if dbg_tap == "h2":
                        sl = se2(w) * RS
                        nc.gpsimd.dma_start(
                            out=dbg[:, w + 1, :],
                            in_=h2w[0:64, sl:sl + RS])
                        nc.gpsimd.dma_start(
                            out=dbg[:, w + 2, :],
                            in_=h2w[64:128, sl:sl + RS])

                v = 2 * k - 14          # conv3 pair (v, v+1), v even
                if 0 <= v <= R - 2:
                    pse = psum.tile([128, 384], F32, tag="c3e",
                                    name=f"c3e_{k}")
                    pso = psum.tile([128, 384], F32, tag="c3o",
                                    name=f"c3o_{k}")
                    emit_conv3(v, pse, pso)
                    if DIRECT_EXP:
                        emit_exp(v, 0, pse)
                        emit_exp(v, 1, pso)
                    else:
                        er = er_pool.tile([81, 768], F16,
                                          name=f"er_{v}", tag="er")
                        emit_exp_er(v, er, 0, pse)
                        emit_exp_er(v, er, 1, pso)
                        emit_scatter(v, er)
                    if (v + 1) % SR == SR - 1:
                        s = v // SR
                        for q in range(4):
                            emit_xbar_q(s, q)
                        if s + 1 < nstrip:
                            emit_rgb_dma(s + 1)
                        emit_bokeh(s)

    nc.compile()
    return nc


# ------------------------- host side -------------------------

def prep_weights(w1, b1, w2, b2, w3, b3, flip=False):
    if flip:
        perm = np.array([(8 - t // 9) * 9 + t % 9 for t in range(81)])
        w1 = w1[:, :, ::-1, :]
        w2 = w2[:, :, ::-1, :]
        w3 = w3[perm][:, :, ::-1, :]
        b3 = b3[perm]
    wtsb = np.zeros((128, WCOLS), np.float32)
    l1 = w1.transpose(2, 3, 1, 0).reshape(36, 64)
    wtsb[0:36, C_L1:C_L1 + 64] = l1
    wtsb[36:72, C_L1 + 64:C_L1 + 128] = l1
    for kw in range(3):
        kh0 = w2[:, :, 0, kw].T
        kh1 = w2[:, :, 1, kw].T
        kh2 = w2[:, :, 2, kw].T
        a0 = C_A + 128 * kw
        wtsb[0:64, a0:a0 + 64] = kh0
        wtsb[64:128, a0:a0 + 64] = kh1
        wtsb[64:128, a0 + 64:a0 + 128] = kh0
        b0 = C_B + 128 * kw
        wtsb[0:64, b0:b0 + 64] = kh2
        wtsb[0:64, b0 + 64:b0 + 128] = kh1
        wtsb[64:128, b0 + 64:b0 + 128] = kh2
        va = C_LVA + 81 * kw
        wtsb[0:64, va:va + 81] = w3[:, :, 1, kw].T
        wtsb[64:128, va:va + 81] = w3[:, :, 2, kw].T
        vb = C_LVB + 81 * kw
        wtsb[0:64, vb:vb + 81] = w3[:, :, 0, kw].T
        wtsb[64:128, vb:vb + 81] = w3[:, :, 1, kw].T
        vs = C_S + 81 * kw
        wtsb[0:64, vs:vs + 81] = w3[:, :, 2, kw].T
        wtsb[64:128, vs:vs + 81] = w3[:, :, 0, kw].T
    wtb = np.zeros((128, 3), np.float32)
    wtb[0:64, 0] = b1
    wtb[64:128, 0] = b1
    wtb[0:64, 1] = b2
    wtb[64:128, 1] = b2
    wtb[0:81, 2] = b3
    return wtsb.astype(np.float16), wtb


def prep_shard(x, rgb_b, R):
    """x: (4,H,W) fp32 of one (possibly flipped) image; rgb_b: (3,H,W).
    Shard = rows 0..R-1 out; top edge is the image edge (zero pad),
    bottom halo rows come from the rest of the image.

    Returns (x36d, rgbs): odd-pair-packed im2col'd conv1 input and
    per-strip rgb halo blocks with partition p = r' + 32*qx."""
    # conv1 rows -1 .. R+2 (nrows = R+4); taps read x rows -2 .. R+3
    nrows = R + 4
    xp = np.zeros((4, R + 8, RS), np.float32)  # image row i at xp[i+3]
    hi = min(R + 4, H)
    xp[:, 3:3 + hi, 1:385] = x[:, 0:hi, :]
    x36f = np.zeros((36, nrows, RS), np.float16)
    for kh in range(3):
        # conv1 row index i (row r = i-1): tap row r+kh-1 -> xp[i+kh+1]
        sl = xp[:, kh + 1:kh + 1 + nrows, :]
        for kw in range(3):
            blk = np.zeros((4, nrows, RS), np.float32)
            if kw == 0:
                blk[:, :, 1:] = sl[:, :, :-1]
            elif kw == 1:
                blk[:, :, :] = sl
            else:
                blk[:, :, :-1] = sl[:, :, 1:]
            for c in range(4):
                x36f[kh * 12 + kw * 4 + c] = blk[c].astype(np.float16)
    # conv1-out row -1 must be exactly zero (image-edge h1 padding)
    x36f[:, 0, :] = 0
    x36d = np.zeros((72, nrows // 2, RS), np.float16)
    x36d[0:36] = x36f[:, 0::2, :]
    x36d[36:72] = x36f[:, 1::2, :]

    # rgb halo rows -4 .. R+4; partition p = r' + 32*qx
    rgbp = np.zeros((3, R + 8, W + 8), np.float32)
    hi2 = min(R + 4, H)
    rgbp[:, 4:4 + hi2, 4:4 + W] = rgb_b[:, 0:hi2, :]
    nstrip = R // SR
    arr = np.zeros((nstrip * 128, RGBF), np.float16)
    rows = rgbp.astype(np.float16)  # (3, R+8, 392)
    for s in range(nstrip):
        for dy in range(9):
            seg = rows[:, s * SR + dy:s * SR + dy + SR, :]  # (3,SR,392)
            for qx in range(4):
                qseg = seg[:, :, qx * 96:qx * 96 + RGBW]  # (3,SR,104)
                view = arr[s * 128 + qx:s * 128 + 128 + qx:4]
                for c in range(3):
                    view[:, (c * 9 + dy) * RGBW:
                         (c * 9 + dy + 1) * RGBW] = qseg[c]
    return x36d, arr


def _prep_inputs(rgb, depth, w1, b1, w2, b2, w3, b3):
    R = H // 2
    x = np.concatenate([rgb, depth], axis=1)  # (B,4,H,W)
    wt_n = prep_weights(w1, b1, w2, b2, w3, b3, flip=False)
    wt_f = prep_weights(w1, b1, w2, b2, w3, b3, flip=True)
    in_maps = []
    for core in range(NC_):
        bi, half = divmod(core, 2)
        if half == 0:
            xi, ri = x[bi], rgb[bi]
            wtsb, wtb = wt_n
        else:
            xi, ri = x[bi, :, ::-1, :], rgb[bi, :, ::-1, :]
            wtsb, wtb = wt_f
        x36d, rgbs = prep_shard(xi, ri, R)
        in_maps.append({"x36d": x36d, "rgbs": rgbs,
                        "wtsb": wtsb, "wtb": wtb})
    return in_maps


_CACHE = {}


def _get_program(R=H // 2, dbg_tap=None):
    key = (R, dbg_tap)
    if key not in _CACHE:
        _CACHE[key] = build_core_program(R, dbg_tap)
    return _CACHE[key]


def kernel(rgb, depth, w1, b1, w2, b2, w3, b3):
    from concourse.bass_utils import run_bass_kernel_spmd
    rgb = np.asarray(rgb, np.float32)
    depth = np.asarray(depth, np.float32)
    nc = _get_program()
    in_maps = _prep_inputs(rgb, depth, np.asarray(w1, np.float32),
                           np.asarray(b1, np.float32),
                           np.asarray(w2, np.float32),
                           np.asarray(b2, np.float32),
                           np.asarray(w3, np.float32),
                           np.asarray(b3, np.float32))
    res = run_bass_kernel_spmd(nc, in_maps, core_ids=list(range(NC_)),
                               trace=bool(int(os.environ.get("KT_TRACE",
                                                             "0"))))
    R = H // 2
    outp = np.zeros((B, 3, H, W), np.float32)
    for core in range(NC_):
        bi, half = divmod(core, 2)
        o = res.results[core]["out"]
        if half == 0:
            outp[bi, :, 0:R, :] = o
        else:
            outp[bi, :, R:H, :] = o[:, ::-1, :]
    kernel.last_result = res
    return outp


if __name__ == "__main__":
    nc = build_core_program(R=int(sys.argv[1]) if len(sys.argv) > 1 else 32)
    print("built ok")


# revision 7
# speedup vs baseline: 2.1860x; 1.3282x over previous
"""DepthAwareBokehDFN Trainium2 kernel, v3.

Per image: x = concat(rgb, depth) (4ch) -> conv3x3(64)+relu ->
conv3x3(64)+relu -> conv3x3(81) -> softmax over 81 taps -> 9x9 dynamic
filtering of rgb.  Data parallel over 8 cores; shard = (batch, H-half),
R=192 output rows per core.  Halos recomputed from DRAM.

v3 dataflow per core (PE-lean row-streamed convs):
  - conv1: host pair-packs im2col input as ODD pairs (rows 2k-1, 2k);
    one K=72 block-diag matmul per pair -> h1w odd slots, no fills.
  - conv2: dense M=128 packing: pair (w, w+1) shares one PSUM tile
    (row w at parts 0:64, w+1 at 64:128); 6 K=128 matmuls/pair with
    75%-dense lhsT blocks A_kw=[[kh0,0],[kh1,kh0]] (rhs = h1w slot w-1)
    and B_kw=[[kh2,kh1],[0,kh2]] (rhs = slot w+1).  ONE relu evac/pair.
  - conv3: 12 matmuls/pair, ALL rhs from native even h2w slots (ZERO
    window-fill DMAs): per pair, rows (v, v+1) share rhs slot v for 6
    K=128 dominoes (lvA=[kh1;kh2] for row v, lvB=[kh0;kh1] for row
    v+1) + 3 K=64 kh0-singles (slot v-2 hi) + 3 K=64 kh2-singles
    (slot v+2 lo).
  - exp ACT -> er (contiguous), DVE scatter -> estg[t, 128*l + p],
    p = 4*r' + qx; one XBAR dma_start_transpose per quarter-strip ->
    eb[p, 96*l + t].
  - bokeh per strip on DVE: tap-product muls, 40/20/10-folds + tap80
    add, reduce, reciprocal normalize.
  - engine/queue placement: XBARs exclusively on SP hwdge; x36
    (prefetched one 8-pair batch ahead), rgb and out DMAs on gpsimd
    swdge.  PSUM: 4 tags x 2 bufs = 8 banks.
"""

import os
import sys
import numpy as np

if "/opt/trn_rl_repo" not in sys.path:
    sys.path.insert(0, "/opt/trn_rl_repo")

import concourse.bass as bass  # noqa: E402
import concourse.bacc as bacc  # noqa: E402
import concourse.mybir as mybir  # noqa: E402
import concourse.tile as tile  # noqa: E402

F32 = mybir.dt.float32
F16 = mybir.dt.float16

B, H, W = 4, 384, 384
NC_ = 8          # cores
RS = 392         # row slot stride (elements) in window buffers
QW = 96          # quarter-row width
SR = 32          # rows per bokeh strip (=> 128 partitions = 32*4)
NS = 8           # slots in h1w window (odd pairs)
NS2 = 8          # slots in h2w window (even pairs)
NSX = 16         # slots in x36 window (8-pair batches, double-buffered)
RGBW = 104       # rgb halo block row width (96 + 8)
RGBF = 27 * RGBW  # rgb halo block elems per partition (3ch * 9dy * 104)
EB = 96 * 96     # eb free elems per partition (l-major, 96 tap slots)
ESF = SR * W     # ESTG free elems (12288)

# weight-table column layout (fp16 table)
C_L1 = 0                  # block-diag conv1 lhsT (72,128)
C_A = 128                 # conv2 dense A_kw: 3x (128,128)
C_B = C_A + 384           # conv2 dense B_kw: 3x (128,128)
C_LVA = C_B + 384         # conv3 row-v dominoes [kh1;kh2]_kw: 3x (128,81)
C_LVB = C_LVA + 243       # conv3 row-v+1 dominoes [kh0;kh1]_kw: 3x (128,81)
C_S = C_LVB + 243         # conv3 singles [kh2 lo; kh0 hi]_kw: 3x (128,81)
WCOLS = C_S + 243

AF = mybir.ActivationFunctionType
ALU = mybir.AluOpType

DIRECT_EXP = True   # exp ACT writes estg directly (else er + DVE scatter)


def build_core_program(R=192, dbg_tap=None):
    """Per-core Bass program.  R = output rows (multiple of SR)."""
    assert R % SR == 0
    nstrip = R // SR
    P1 = (R + 4) // 2   # conv1 pairs (rows -1 .. R+2)

    nc = bacc.Bacc("TRN2", debug=False, enable_asserts=False,
                   num_devices=NC_, enable_partition_id=False,
                   num_swdge_queues=4)

    x36d = nc.dram_tensor("x36d", [72, P1, RS], F16,
                          kind="ExternalInput").ap()
    rgbs = nc.dram_tensor("rgbs", [nstrip * 128, RGBF], F16,
                          kind="ExternalInput").ap()
    wtsb = nc.dram_tensor("wtsb", [128, WCOLS], F16,
                          kind="ExternalInput").ap()
    wtb = nc.dram_tensor("wtb", [128, 3], F32, kind="ExternalInput").ap()
    out = nc.dram_tensor("out", [3, R, W], F32, kind="ExternalOutput").ap()
    dbg = None
    if dbg_tap == "E":
        dbg = nc.dram_tensor("dbg", [nstrip, 96, ESF], F16,
                             kind="ExternalOutput").ap()
    elif dbg_tap == "eb":
        dbg = nc.dram_tensor("dbg", [nstrip, 128, EB], F16,
                             kind="ExternalOutput").ap()
    elif dbg_tap == "h1":
        dbg = nc.dram_tensor("dbg", [64, R + 4, RS], F16,
                             kind="ExternalOutput").ap()
    elif dbg_tap == "h2":
        dbg = nc.dram_tensor("dbg", [64, R + 4, RS], F16,
                             kind="ExternalOutput").ap()

    def so1(j):  # h1w slot of odd row j (pair (j, j+1)), j = -1, 1, 3, ...
        return ((j + 1) // 2) % NS

    def se2(w):  # h2w slot of even conv2 pair (w, w+1), w = -2, 0, 2, ...
        return (w // 2 + 1) % NS2

    with tile.TileContext(nc) as tc:
        with (
            tc.tile_pool(name="singles", bufs=1) as singles,
            tc.tile_pool(name="ostg_pool", bufs=2) as ostg_pool,
            tc.tile_pool(name="er_pool", bufs=4) as er_pool,
            tc.tile_pool(name="psum", bufs=2, space="PSUM") as psum,
        ):
            # ---- persistent SBUF state ----
            wtsb_sb = singles.tile([128, WCOLS], F16)
            nc.sync.dma_start(out=wtsb_sb, in_=wtsb)
            wtb_sb = singles.tile([128, 3], F32)
            nc.sync.dma_start(out=wtb_sb, in_=wtb)
            h1w = singles.tile([128, NS * RS], F16)
            h2w = singles.tile([128, NS2 * RS], F16)
            x36 = singles.tile([72, NSX * RS], F16)
            estg = [singles.tile([96, ESF], F16, name=f"estg{i}")
                    for i in range(2)]
            eb = [singles.tile([128, EB], F16, name=f"eb{i}")
                  for i in range(2)]
            rgbb = [singles.tile([128, RGBF], F16, name=f"rgbb{i}")
                    for i in range(2)]
            tmpP = singles.tile([128, 96 * 81], F16)
            sA40 = singles.tile([128, 40 * QW], F16)
            s20 = singles.tile([128, 20 * QW], F16)
            s10 = singles.tile([128, 10 * QW], F16)
            uacc = singles.tile([128, 4, QW], F32)

            nc.vector.memset(h1w, 0.0)
            nc.vector.memset(h2w, 0.0)
            nc.vector.memset(x36, 0.0)
            for es_ in estg:
                nc.vector.memset(es_[0:96, :], 0.0)

            # weight slices
            l1 = wtsb_sb[0:72, C_L1:C_L1 + 128]
            wA = [wtsb_sb[0:128, C_A + 128 * k:C_A + 128 * (k + 1)]
                  for k in range(3)]
            wB = [wtsb_sb[0:128, C_B + 128 * k:C_B + 128 * (k + 1)]
                  for k in range(3)]
            lvA = [wtsb_sb[0:128, C_LVA + 81 * k:C_LVA + 81 * (k + 1)]
                   for k in range(3)]
            lvB = [wtsb_sb[0:128, C_LVB + 81 * k:C_LVB + 81 * (k + 1)]
                   for k in range(3)]
            sg2 = [wtsb_sb[0:64, C_S + 81 * k:C_S + 81 * (k + 1)]
                   for k in range(3)]
            sg0 = [wtsb_sb[64:128, C_S + 81 * k:C_S + 81 * (k + 1)]
                   for k in range(3)]

            def bias(col, lo, hi):
                return wtb_sb[lo:hi, col:col + 1]

            # ---------------- emission helpers ----------------
            def emit_x36_batch(k0):
                # prefetch pair-slots k0 .. k0+7 (window slot = k % NSX)
                n = min(8, P1 - k0)
                if n <= 0:
                    return
                F = NSX * RS
                dst = bass.AP(tensor=x36.tensor, offset=(k0 % NSX) * RS,
                              ap=[[F, 72], [RS, n], [1, RS]])
                src = bass.AP(tensor=x36d.tensor, offset=k0 * RS,
                              ap=[[P1 * RS, 72], [RS, n], [1, RS]])
                nc.gpsimd.dma_start(out=dst, in_=src)

            def emit_conv1(k, ps):
                # one K=72 block-diag matmul: psum 0:64 = row 2k-1,
                # 64:128 = row 2k
                rhs = x36[0:72, (k % NSX) * RS + 1:(k % NSX) * RS + 385]
                nc.tensor.matmul(out=ps[0:128, 0:384], lhsT=l1, rhs=rhs,
                                 start=True, stop=True)

            def emit_conv2(w, ps):
                # dense M=128: rows (w, w+1) in one PSUM tile
                sa = so1(w - 1) * RS
                sb = so1(w + 1) * RS
                for kw in range(3):
                    nc.tensor.matmul(out=ps[0:128, 0:384], lhsT=wA[kw],
                                     rhs=h1w[0:128, sa + kw:sa + kw + 384],
                                     start=(kw == 0), stop=False)
                for kw in range(3):
                    nc.tensor.matmul(out=ps[0:128, 0:384], lhsT=wB[kw],
                                     rhs=h1w[0:128, sb + kw:sb + kw + 384],
                                     start=False, stop=(kw == 2))

            def emit_conv2_evac(w, ps):
                sl = se2(w) * RS
                nc.scalar.activation(
                    out=h2w[0:128, sl + 1:sl + 385],
                    in_=ps[0:128, 0:384], func=AF.Relu, bias=bias(1, 0, 128))

            def emit_conv3(v, pse, pso):
                # 6 K=128 dominoes sharing rhs slot v (lvA: row v gets
                # kh1,kh2; lvB: row v+1 gets kh0,kh1) + 3 K=64
                # kh0-singles (slot v-2 hi = row v-1) + 3 K=64
                # kh2-singles (slot v+2 lo = row v+2)
                sv = se2(v) * RS
                sm = se2(v - 2) * RS
                sp = se2(v + 2) * RS
                for kw in range(3):
                    rhs = h2w[0:128, sv + kw:sv + kw + 384]
                    nc.tensor.matmul(out=pse[0:81, 0:384], lhsT=lvA[kw],
                                     rhs=rhs, start=(kw == 0), stop=False)
                    nc.tensor.matmul(out=pso[0:81, 0:384], lhsT=lvB[kw],
                                     rhs=rhs, start=(kw == 0), stop=False)
                for kw in range(3):
                    nc.tensor.matmul(out=pse[0:81, 0:384], lhsT=sg0[kw],
                                     rhs=h2w[64:128, sm + kw:sm + kw + 384],
                                     start=False, stop=(kw == 2))
                    nc.tensor.matmul(out=pso[0:81, 0:384], lhsT=sg2[kw],
                                     rhs=h2w[0:64, sp + kw:sp + kw + 384],
                                     start=False, stop=(kw == 2))

            def emit_exp(v, i, ps):
                # exp(logits+b3) of row v+i
                es = estg[(v // SR) % 2]
                r = v % SR + i
                if DIRECT_EXP:
                    # direct scattered write: estg[t, 128*l + 4*r + qx]
                    dst = bass.AP(tensor=es.tensor, offset=4 * r,
                                  ap=[[ESF, 81], [128, 96], [1, 4]])
                    src = bass.AP(tensor=ps.tensor, offset=0,
                                  ap=[[384, 81], [1, 96], [96, 4]])
                    nc.scalar.activation(out=dst, in_=src, func=AF.Exp,
                                         bias=bias(2, 0, 81))
                    return None
                return ps  # caller stages via er

            def emit_exp_er(v, er, i, ps):
                nc.scalar.activation(out=er[0:81, 384 * i:384 * (i + 1)],
                                     in_=ps[0:81, 0:384],
                                     func=AF.Exp, bias=bias(2, 0, 81))

            def emit_scatter(v, er):
                # DVE scatter into estg[t, 128*l + 4*r + qx]; (i, l, qx)
                # iteration keeps dst runs 4-elem contiguous
                es = estg[(v // SR) % 2]
                r = v % SR
                dst = bass.AP(tensor=es.tensor, offset=4 * r,
                              ap=[[ESF, 81], [4, 2], [128, 96], [1, 4]])
                src = bass.AP(tensor=er.tensor, offset=0,
                              ap=[[768, 81], [384, 2], [1, 96], [96, 4]])
                nc.vector.tensor_copy(out=dst, in_=src)

            def emit_xbar_q(s, q):
                # quarter-strip transpose: eb[p, 96*l + t] for l in 24q..
                es = estg[s % 2]
                dst = bass.AP(tensor=eb[s % 2].tensor, offset=q * 24 * 96,
                              ap=[[EB, 128], [96, 24], [1, 96]])
                nc.sync.dma_start_transpose(
                    out=dst, in_=es[0:96, q * 3072:(q + 1) * 3072])
                if dbg_tap == "E" and q == 0:
                    nc.gpsimd.dma_start(out=dbg[s], in_=es[0:96, :])

            def emit_rgb_dma(s):
                nc.gpsimd.dma_start(
                    out=rgbb[s % 2][0:128, :],
                    in_=rgbs[s * 128:(s + 1) * 128, :])

            def emit_bokeh(s):
                if dbg_tap == "eb":
                    nc.gpsimd.dma_start(out=dbg[s], in_=eb[s % 2][0:128, :])
                ostg = ostg_pool.tile([128, 3, QW], F32, name=f"ostg{s}",
                                      tag="ostg")
                with nc.allow_low_precision("fp16 bokeh by design"):
                    for ch in range(4):
                        if ch < 3:
                            # tmpP[(l,dy,dx)] = E * rgb_shift  (tap-minor)
                            dst = bass.AP(
                                tensor=tmpP.tensor, offset=0,
                                ap=[[96 * 81, 128], [81, 96], [9, 9],
                                    [1, 9]])
                            ein = bass.AP(
                                tensor=eb[s % 2].tensor, offset=0,
                                ap=[[EB, 128], [96, 96], [9, 9], [1, 9]])
                            rin = bass.AP(
                                tensor=rgbb[s % 2].tensor,
                                offset=ch * 9 * RGBW,
                                ap=[[RGBF, 128], [1, 96], [RGBW, 9],
                                    [1, 9]])
                            nc.vector.tensor_mul(dst, ein, rin)
                            src_t, tst = tmpP, 81
                        else:
                            src_t, tst = eb[s % 2], 96
                        srcf = 96 * tst

                        def sap(off, cnt):
                            return bass.AP(tensor=src_t.tensor,
                                           offset=off,
                                           ap=[[srcf, 128], [tst, 96],
                                               [1, cnt]])

                        def a40(cnt, t0=0):
                            return bass.AP(tensor=sA40.tensor,
                                           offset=t0,
                                           ap=[[40 * QW, 128],
                                               [40, 96], [1, cnt]])

                        def s20ap(cnt, t0=0):
                            return bass.AP(tensor=s20.tensor, offset=t0,
                                           ap=[[20 * QW, 128], [20, 96],
                                               [1, cnt]])

                        def s10ap(cnt, t0=0):
                            return bass.AP(tensor=s10.tensor, offset=t0,
                                           ap=[[10 * QW, 128], [10, 96],
                                               [1, cnt]])

                        # 80->40 fold, then add tap 80 into column 0
                        nc.vector.tensor_add(a40(40), sap(0, 40),
                                             sap(40, 40))
                        nc.vector.tensor_add(
                            bass.AP(tensor=sA40.tensor, offset=0,
                                    ap=[[40 * QW, 128], [40, 96]]),
                            bass.AP(tensor=sA40.tensor, offset=0,
                                    ap=[[40 * QW, 128], [40, 96]]),
                            bass.AP(tensor=src_t.tensor, offset=80,
                                    ap=[[srcf, 128], [tst, 96]]))
                        nc.vector.tensor_add(s20ap(20), a40(20),
                                             a40(20, t0=20))
                        nc.vector.tensor_add(s10ap(10), s20ap(10),
                                             s20ap(10, t0=10))
                        nc.vector.tensor_reduce(
                            out=uacc[0:128, ch, :], in_=s10ap(10),
                            axis=mybir.AxisListType.X, op=ALU.add)

                    nc.vector.reciprocal(uacc[0:128, 3, :],
                                         uacc[0:128, 3, :])
                    for ch in range(3):
                        nc.vector.tensor_mul(ostg[0:128, ch, :],
                                             uacc[0:128, ch, :],
                                             uacc[0:128, 3, :])

                for ch in range(3):
                    dst = bass.AP(tensor=out.tensor,
                                  offset=ch * R * W + s * SR * W,
                                  ap=[[W, SR], [QW, 4], [1, QW]])
                    src = bass.AP(tensor=ostg.tensor, offset=ch * QW,
                                  ap=[[3 * QW, 128], [1, QW]])
                    nc.gpsimd.dma_start(out=dst, in_=src)

            # ---------------- main row loop ----------------
            emit_rgb_dma(0)
            emit_x36_batch(0)
            kmax = R // 2 + 6
            for k in range(kmax + 1):
                j1 = 2 * k - 1          # conv1 pair (j1, j1+1)
                if j1 <= R + 1:
                    if k % 8 == 0:
                        emit_x36_batch(k + 8)
                    ps1 = psum.tile([128, 384], F32, tag="c1",
                                    name=f"c1_{k}")
                    emit_conv1(k, ps1)
                    nc.scalar.activation(
                        out=h1w[0:128,
                                so1(j1) * RS + 1:so1(j1) * RS + 385],
                        in_=ps1[0:128, 0:384], func=AF.Relu,
                        bias=bias(0, 0, 128))
                    if dbg_tap == "h1":
                        sl = so1(j1) * RS
                        nc.gpsimd.dma_start(
                            out=dbg[:, j1 + 1, :],
                            in_=h1w[0:64, sl:sl + RS])
                        nc.gpsimd.dma_start(
                            out=dbg[:, j1 + 2, :],
                            in_=h1w[64:128, sl:sl + RS])

                w = 2 * k - 8           # conv2 pair (w, w+1), w even
                if 0 <= w <= R:
                    ps2 = psum.tile([128, 384], F32, tag="c2",
                                    name=f"c2_{k}")
                    emit_conv2(w, ps2)
                    emit_conv2_evac(w, ps2)
                    if dbg_tap == "h2":
                        sl = se2(w) * RS
                        nc.gpsimd.dma_start(
                            out=dbg[:, w + 1, :],
                            in_=h2w[0:64, sl:sl + RS])
                        nc.gpsimd.dma_start(
                            out=dbg[:, w + 2, :],
                            in_=h2w[64:128, sl:sl + RS])

                v = 2 * k - 14          # conv3 pair (v, v+1), v even
                if 0 <= v <= R - 2:
                    pse = psum.tile([128, 384], F32, tag="c3e",
                                    name=f"c3e_{k}")
                    pso = psum.tile([128, 384], F32, tag="c3o",
                                    name=f"c3o_{k}")
                    emit_conv3(v, pse, pso)
                    if DIRECT_EXP:
                        emit_exp(v, 0, pse)
                        emit_exp(v, 1, pso)
                    else:
                        er = er_pool.tile([81, 768], F16,
                                          name=f"er_{v}", tag="er")
                        emit_exp_er(v, er, 0, pse)
                        emit_exp_er(v, er, 1, pso)
                        emit_scatter(v, er)
                    if (v + 1) % SR == SR - 1:
                        s = v // SR
                        for q in range(4):
                            emit_xbar_q(s, q)
                        if s + 1 < nstrip:
                            emit_rgb_dma(s + 1)
                        emit_bokeh(s)

    nc.compile()
    return nc


# ------------------------- host side -------------------------

def prep_weights(w1, b1, w2, b2, w3, b3, flip=False):
    if flip:
        perm = np.array([(8 - t // 9) * 9 + t % 9 for t in range(81)])
        w1 = w1[:, :, ::-1, :]
        w2 = w2[:, :, ::-1, :]
        w3 = w3[perm][:, :, ::-1, :]
        b3 = b3[perm]
    wtsb = np.zeros((128, WCOLS), np.float32)
    l1 = w1.transpose(2, 3, 1, 0).reshape(36, 64)
    wtsb[0:36, C_L1:C_L1 + 64] = l1
    wtsb[36:72, C_L1 + 64:C_L1 + 128] = l1
    for kw in range(3):
        kh0 = w2[:, :, 0, kw].T
        kh1 = w2[:, :, 1, kw].T
        kh2 = w2[:, :, 2, kw].T
        a0 = C_A + 128 * kw
        wtsb[0:64, a0:a0 + 64] = kh0
        wtsb[64:128, a0:a0 + 64] = kh1
        wtsb[64:128, a0 + 64:a0 + 128] = kh0
        b0 = C_B + 128 * kw
        wtsb[0:64, b0:b0 + 64] = kh2
        wtsb[0:64, b0 + 64:b0 + 128] = kh1
        wtsb[64:128, b0 + 64:b0 + 128] = kh2
        va = C_LVA + 81 * kw
        wtsb[0:64, va:va + 81] = w3[:, :, 1, kw].T
        wtsb[64:128, va:va + 81] = w3[:, :, 2, kw].T
        vb = C_LVB + 81 * kw
        wtsb[0:64, vb:vb + 81] = w3[:, :, 0, kw].T
        wtsb[64:128, vb:vb + 81] = w3[:, :, 1, kw].T
        vs = C_S + 81 * kw
        wtsb[0:64, vs:vs + 81] = w3[:, :, 2, kw].T
        wtsb[64:128, vs:vs + 81] = w3[:, :, 0, kw].T
    wtb = np.zeros((128, 3), np.float32)
    wtb[0:64, 0] = b1
    wtb[64:128, 0] = b1
    wtb[0:64, 1] = b2
    wtb[64:128, 1] = b2
    wtb[0:81, 2] = b3
    return wtsb.astype(np.float16), wtb


def prep_shard(x, rgb_b, R):
    """x: (4,H,W) fp32 of one (possibly flipped) image; rgb_b: (3,H,W).
    Shard = rows 0..R-1 out; top edge is the image edge (zero pad),
    bottom halo rows come from the rest of the image.

    Returns (x36d, rgbs): odd-pair-packed im2col'd conv1 input and
    per-strip rgb halo blocks with partition p = r' + 32*qx."""
    # conv1 rows -1 .. R+2 (nrows = R+4); taps read x rows -2 .. R+3
    nrows = R + 4
    xp = np.zeros((4, R + 8, RS), np.float32)  # image row i at xp[i+3]
    hi = min(R + 4, H)
    xp[:, 3:3 + hi, 1:385] = x[:, 0:hi, :]
    x36f = np.zeros((36, nrows, RS), np.float16)
    for kh in range(3):
        # conv1 row index i (row r = i-1): tap row r+kh-1 -> xp[i+kh+1]
        sl = xp[:, kh + 1:kh + 1 + nrows, :]
        for kw in range(3):
            blk = np.zeros((4, nrows, RS), np.float32)
            if kw == 0:
                blk[:, :, 1:] = sl[:, :, :-1]
            elif kw == 1:
                blk[:, :, :] = sl
            else:
                blk[:, :, :-1] = sl[:, :, 1:]
            for c in range(4):
                x36f[kh * 12 + kw * 4 + c] = blk[c].astype(np.float16)
    # conv1-out row -1 must be exactly zero (image-edge h1 padding)
    x36f[:, 0, :] = 0
    x36d = np.zeros((72, nrows // 2, RS), np.float16)
    x36d[0:36] = x36f[:, 0::2, :]
    x36d[36:72] = x36f[:, 1::2, :]

    # rgb halo rows -4 .. R+4; partition p = r' + 32*qx
    rgbp = np.zeros((3, R + 8, W + 8), np.float32)
    hi2 = min(R + 4, H)
    rgbp[:, 4:4 + hi2, 4:4 + W] = rgb_b[:, 0:hi2, :]
    nstrip = R // SR
    arr = np.zeros((nstrip * 128, RGBF), np.float16)
    rows = rgbp.astype(np.float16)  # (3, R+8, 392)
    for s in range(nstrip):
        for dy in range(9):
            seg = rows[:, s * SR + dy:s * SR + dy + SR, :]  # (3,SR,392)
            for qx in range(4):
                qseg = seg[:, :, qx * 96:qx * 96 + RGBW]  # (3,SR,104)
                view = arr[s * 128 + qx:s * 128 + 128 + qx:4]
                for c in range(3):
                    view[:, (c * 9 + dy) * RGBW:
                         (c * 9 + dy + 1) * RGBW] = qseg[c]
    return x36d, arr


def _prep_inputs(rgb, depth, w1, b1, w2, b2, w3, b3):
    R = H // 2
    x = np.concatenate([rgb, depth], axis=1)  # (B,4,H,W)
    wt_n = prep_weights(w1, b1, w2, b2, w3, b3, flip=False)
    wt_f = prep_weights(w1, b1, w2, b2, w3, b3, flip=True)
    in_maps = []
    for core in range(NC_):
        bi, half = divmod(core, 2)
        if half == 0:
            xi, ri = x[bi], rgb[bi]
            wtsb, wtb = wt_n
        else:
            xi, ri = x[bi, :, ::-1, :], rgb[bi, :, ::-1, :]
            wtsb, wtb = wt_f
        x36d, rgbs = prep_shard(xi, ri, R)
        in_maps.append({"x36d": x36d, "rgbs": rgbs,
                        "wtsb": wtsb, "wtb": wtb})
    return in_maps


_CACHE = {}


def _get_program(R=H // 2, dbg_tap=None):
    key = (R, dbg_tap)
    if key not in _CACHE:
        _CACHE[key] = build_core_program(R, dbg_tap)
    return _CACHE[key]


def kernel(rgb, depth, w1, b1, w2, b2, w3, b3):
    from concourse.bass_utils import run_bass_kernel_spmd
    rgb = np.asarray(rgb, np.float32)
    depth = np.asarray(depth, np.float32)
    nc = _get_program()
    in_maps = _prep_inputs(rgb, depth, np.asarray(w1, np.float32),
                           np.asarray(b1, np.float32),
                           np.asarray(w2, np.float32),
                           np.asarray(b2, np.float32),
                           np.asarray(w3, np.float32),
                           np.asarray(b3, np.float32))
    res = run_bass_kernel_spmd(nc, in_maps, core_ids=list(range(NC_)),
                               trace=bool(int(os.environ.get("KT_TRACE",
                                                             "0"))))
    R = H // 2
    outp = np.zeros((B, 3, H, W), np.float32)
    for core in range(NC_):
        bi, half = divmod(core, 2)
        o = res.results[core]["out"]
        if half == 0:
            outp[bi, :, 0:R, :] = o
        else:
            outp[bi, :, R:H, :] = o[:, ::-1, :]
    kernel.last_result = res
    return outp


if __name__ == "__main__":
    nc = build_core_program(R=int(sys.argv[1]) if len(sys.argv) > 1 else 32)
    print("built ok")


# revision 12
# speedup vs baseline: 2.2080x; 1.0101x over previous
"""DepthAwareBokehDFN Trainium2 kernel, v3.

Per image: x = concat(rgb, depth) (4ch) -> conv3x3(64)+relu ->
conv3x3(64)+relu -> conv3x3(81) -> softmax over 81 taps -> 9x9 dynamic
filtering of rgb.  Data parallel over 8 cores; shard = (batch, H-half),
R=192 output rows per core.  Halos recomputed from DRAM.

v3 dataflow per core (PE-lean row-streamed convs):
  - conv1: host pair-packs im2col input as ODD pairs (rows 2k-1, 2k);
    one K=72 block-diag matmul per pair -> h1w odd slots, no fills.
  - conv2: dense M=128 packing: pair (w, w+1) shares one PSUM tile
    (row w at parts 0:64, w+1 at 64:128); 6 K=128 matmuls/pair with
    75%-dense lhsT blocks A_kw=[[kh0,0],[kh1,kh0]] (rhs = h1w slot w-1)
    and B_kw=[[kh2,kh1],[0,kh2]] (rhs = slot w+1).  ONE relu evac/pair.
  - conv3: 12 matmuls/pair, ALL rhs from native even h2w slots (ZERO
    window-fill DMAs): per pair, rows (v, v+1) share rhs slot v for 6
    K=128 dominoes (lvA=[kh1;kh2] for row v, lvB=[kh0;kh1] for row
    v+1) + 3 K=64 kh0-singles (slot v-2 hi) + 3 K=64 kh2-singles
    (slot v+2 lo).
  - exp ACT -> er (contiguous), DVE scatter -> estg[t, 128*l + p],
    p = 4*r' + qx; one XBAR dma_start_transpose per quarter-strip ->
    eb[p, 96*l + t].
  - bokeh per strip on DVE: tap-product muls, 40/20/10-folds + tap80
    add, reduce, reciprocal normalize.
  - engine/queue placement: XBARs exclusively on SP hwdge; x36
    (prefetched one 8-pair batch ahead), rgb and out DMAs on gpsimd
    swdge.  PSUM: 4 tags x 2 bufs = 8 banks.
"""

import os
import sys
import numpy as np

if "/opt/trn_rl_repo" not in sys.path:
    sys.path.insert(0, "/opt/trn_rl_repo")

import concourse.bass as bass  # noqa: E402
import concourse.bacc as bacc  # noqa: E402
import concourse.mybir as mybir  # noqa: E402
import concourse.tile as tile  # noqa: E402

F32 = mybir.dt.float32
F16 = mybir.dt.float16

B, H, W = 4, 384, 384
NC_ = 8          # cores
RS = 392         # row slot stride (elements) in window buffers
QW = 96          # quarter-row width
SR = 32          # rows per bokeh strip (=> 128 partitions = 32*4)
NS = 8           # slots in h1w window (odd pairs)
NS2 = 8          # slots in h2w window (even pairs)
NSX = 16         # slots in x36 window (8-pair batches, double-buffered)
RGBW = 104       # rgb halo block row width (96 + 8)
RGBF = 27 * RGBW  # rgb halo block elems per partition (3ch * 9dy * 104)
EB = 96 * 96     # eb free elems per partition (l-major, 96 tap slots)
ESF = SR * W     # ESTG free elems (12288)

# weight-table column layout (fp16 table)
C_L1 = 0                  # block-diag conv1 lhsT (72,128)
C_A = 128                 # conv2 dense A_kw: 3x (128,128)
C_B = C_A + 384           # conv2 dense B_kw: 3x (128,128)
C_LVA = C_B + 384         # conv3 row-v dominoes [kh1;kh2]_kw: 3x (128,81)
C_LVB = C_LVA + 243       # conv3 row-v+1 dominoes [kh0;kh1]_kw: 3x (128,81)
C_S = C_LVB + 243         # conv3 singles [kh2 lo; kh0 hi]_kw: 3x (128,81)
WCOLS = C_S + 243

AF = mybir.ActivationFunctionType
ALU = mybir.AluOpType

DIRECT_EXP = True   # exp ACT writes estg directly (else er + DVE scatter)


def build_core_program(R=192, dbg_tap=None):
    """Per-core Bass program.  R = output rows (multiple of SR)."""
    assert R % SR == 0
    nstrip = R // SR
    P1 = (R + 4) // 2   # conv1 pairs (rows -1 .. R+2)

    nc = bacc.Bacc("TRN2", debug=False, enable_asserts=False,
                   num_devices=NC_, enable_partition_id=False,
                   num_swdge_queues=4)

    x36d = nc.dram_tensor("x36d", [72, P1, RS], F16,
                          kind="ExternalInput").ap()
    rgbs = nc.dram_tensor("rgbs", [nstrip * 128, RGBF], F16,
                          kind="ExternalInput").ap()
    wtsb = nc.dram_tensor("wtsb", [128, WCOLS], F16,
                          kind="ExternalInput").ap()
    wtb = nc.dram_tensor("wtb", [128, 3], F32, kind="ExternalInput").ap()
    out = nc.dram_tensor("out", [3, R, W], F32, kind="ExternalOutput").ap()
    dbg = None
    if dbg_tap == "E":
        dbg = nc.dram_tensor("dbg", [nstrip, 96, ESF], F16,
                             kind="ExternalOutput").ap()
    elif dbg_tap == "eb":
        dbg = nc.dram_tensor("dbg", [nstrip, 128, EB], F16,
                             kind="ExternalOutput").ap()
    elif dbg_tap == "h1":
        dbg = nc.dram_tensor("dbg", [64, R + 4, RS], F16,
                             kind="ExternalOutput").ap()
    elif dbg_tap == "h2":
        dbg = nc.dram_tensor("dbg", [64, R + 4, RS], F16,
                             kind="ExternalOutput").ap()

    def so1(j):  # h1w slot of odd row j (pair (j, j+1)), j = -1, 1, 3, ...
        return ((j + 1) // 2) % NS

    def se2(w):  # h2w slot of even conv2 pair (w, w+1), w = -2, 0, 2, ...
        return (w // 2 + 1) % NS2

    with tile.TileContext(nc) as tc:
        with (
            tc.tile_pool(name="singles", bufs=1) as singles,
            tc.tile_pool(name="ostg_pool", bufs=2) as ostg_pool,
            tc.tile_pool(name="er_pool", bufs=4) as er_pool,
            tc.tile_pool(name="psum", bufs=2, space="PSUM") as psum,
        ):
            # ---- persistent SBUF state ----
            wtsb_sb = singles.tile([128, WCOLS], F16)
            nc.sync.dma_start(out=wtsb_sb, in_=wtsb)
            wtb_sb = singles.tile([128, 3], F32)
            nc.sync.dma_start(out=wtb_sb, in_=wtb)
            h1w = singles.tile([128, NS * RS], F16)
            h2w = singles.tile([128, NS2 * RS], F16)
            x36 = singles.tile([128, NSX * RS], F16)
            estg = [singles.tile([96, ESF], F16, name=f"estg{i}")
                    for i in range(2)]
            eb = [singles.tile([128, EB], F16, name=f"eb{i}")
                  for i in range(2)]
            rgbb = [singles.tile([128, RGBF], F16, name=f"rgbb{i}")
                    for i in range(2)]
            tmpP = singles.tile([128, 96 * 81], F16)
            sA40 = singles.tile([128, 40 * QW], F16)
            s20 = singles.tile([128, 20 * QW], F16)
            s10 = singles.tile([128, 10 * QW], F16)
            uacc = singles.tile([128, 4, QW], F32)

            nc.vector.memset(x36[64:128, :], 0.0)
            nc.vector.memset(h1w, 0.0)
            nc.vector.memset(h2w, 0.0)
            for es_ in estg:
                nc.vector.memset(es_[0:96, :], 0.0)

            # weight slices (conv1 lhsT padded to K=128: table rows
            # 72:128 are zero, so garbage x36 partitions 72:128 and any
            # stale PE rows are multiplied by zero weights)
            l1 = wtsb_sb[0:128, C_L1:C_L1 + 128]
            wA = [wtsb_sb[0:128, C_A + 128 * k:C_A + 128 * (k + 1)]
                  for k in range(3)]
            wB = [wtsb_sb[0:128, C_B + 128 * k:C_B + 128 * (k + 1)]
                  for k in range(3)]
            lvA = [wtsb_sb[0:128, C_LVA + 81 * k:C_LVA + 81 * (k + 1)]
                   for k in range(3)]
            lvB = [wtsb_sb[0:128, C_LVB + 81 * k:C_LVB + 81 * (k + 1)]
                   for k in range(3)]
            sg2 = [wtsb_sb[0:64, C_S + 81 * k:C_S + 81 * (k + 1)]
                   for k in range(3)]
            sg0 = [wtsb_sb[64:128, C_S + 81 * k:C_S + 81 * (k + 1)]
                   for k in range(3)]

            def bias(col, lo, hi):
                return wtb_sb[lo:hi, col:col + 1]

            # ---------------- emission helpers ----------------
            def emit_x36_batch(k0):
                # prefetch pair-slots k0 .. k0+7 (window slot = k % NSX)
                n = min(8, P1 - k0)
                if n <= 0:
                    return
                F = NSX * RS
                dst = bass.AP(tensor=x36.tensor, offset=(k0 % NSX) * RS,
                              ap=[[F, 72], [RS, n], [1, RS]])
                srcp = bass.AP(tensor=x36d.tensor, offset=k0 * RS,
                               ap=[[P1 * RS, 72], [RS, n], [1, RS]])
                nc.gpsimd.dma_start(out=dst, in_=srcp)

            def emit_conv1(k, ps):
                # one K=72 block-diag matmul: psum 0:64 = row 2k-1,
                # 64:128 = row 2k
                rhs = x36[0:128, (k % NSX) * RS + 1:(k % NSX) * RS + 385]
                nc.tensor.matmul(out=ps[0:128, 0:384], lhsT=l1, rhs=rhs,
                                 start=True, stop=True)

            def emit_conv2(w, ps):
                # dense M=128: rows (w, w+1) in one PSUM tile
                sa = so1(w - 1) * RS
                sb = so1(w + 1) * RS
                for kw in range(3):
                    nc.tensor.matmul(out=ps[0:128, 0:384], lhsT=wA[kw],
                                     rhs=h1w[0:128, sa + kw:sa + kw + 384],
                                     start=(kw == 0), stop=False)
                for kw in range(3):
                    nc.tensor.matmul(out=ps[0:128, 0:384], lhsT=wB[kw],
                                     rhs=h1w[0:128, sb + kw:sb + kw + 384],
                                     start=False, stop=(kw == 2))

            def emit_conv2_evac(w, ps):
                sl = se2(w) * RS
                nc.scalar.activation(
                    out=h2w[0:128, sl + 1:sl + 385],
                    in_=ps[0:128, 0:384], func=AF.Relu, bias=bias(1, 0, 128))

            def emit_conv3(v, pse, pso):
                # 6 K=128 dominoes sharing rhs slot v (lvA: row v gets
                # kh1,kh2; lvB: row v+1 gets kh0,kh1) + 3 K=64
                # kh0-singles (slot v-2 hi = row v-1) + 3 K=64
                # kh2-singles (slot v+2 lo = row v+2)
                sv = se2(v) * RS
                sm = se2(v - 2) * RS
                sp = se2(v + 2) * RS
                for kw in range(3):
                    rhs = h2w[0:128, sv + kw:sv + kw + 384]
                    nc.tensor.matmul(out=pse[0:81, 0:384], lhsT=lvA[kw],
                                     rhs=rhs, start=(kw == 0), stop=False)
                    nc.tensor.matmul(out=pso[0:81, 0:384], lhsT=lvB[kw],
                                     rhs=rhs, start=(kw == 0), stop=False)
                for kw in range(3):
                    nc.tensor.matmul(out=pse[0:81, 0:384], lhsT=sg0[kw],
                                     rhs=h2w[64:128, sm + kw:sm + kw + 384],
                                     start=False, stop=(kw == 2))
                    nc.tensor.matmul(out=pso[0:81, 0:384], lhsT=sg2[kw],
                                     rhs=h2w[0:64, sp + kw:sp + kw + 384],
                                     start=False, stop=(kw == 2))

            def emit_exp(v, i, ps):
                # exp(logits+b3) of row v+i
                es = estg[(v // SR) % 2]
                r = v % SR + i
                if DIRECT_EXP:
                    # direct scattered write: estg[t, 128*l + 4*r + qx]
                    dst = bass.AP(tensor=es.tensor, offset=4 * r,
                                  ap=[[ESF, 81], [128, 96], [1, 4]])
                    src = bass.AP(tensor=ps.tensor, offset=0,
                                  ap=[[384, 81], [1, 96], [96, 4]])
                    nc.scalar.activation(out=dst, in_=src, func=AF.Exp,
                                         bias=bias(2, 0, 81))
                    return None
                return ps  # caller stages via er

            def emit_exp_er(v, er, i, ps):
                nc.scalar.activation(out=er[0:81, 384 * i:384 * (i + 1)],
                                     in_=ps[0:81, 0:384],
                                     func=AF.Exp, bias=bias(2, 0, 81))

            def emit_scatter(v, er):
                # DVE scatter into estg[t, 128*l + 4*r + qx]; (i, l, qx)
                # iteration keeps dst runs 4-elem contiguous
                es = estg[(v // SR) % 2]
                r = v % SR
                dst = bass.AP(tensor=es.tensor, offset=4 * r,
                              ap=[[ESF, 81], [4, 2], [128, 96], [1, 4]])
                src = bass.AP(tensor=er.tensor, offset=0,
                              ap=[[768, 81], [384, 2], [1, 96], [96, 4]])
                nc.vector.tensor_copy(out=dst, in_=src)

            def emit_xbar_q(s, q):
                # quarter-strip transpose: eb[p, 96*l + t] for l in 24q..
                es = estg[s % 2]
                dst = bass.AP(tensor=eb[s % 2].tensor, offset=q * 24 * 96,
                              ap=[[EB, 128], [96, 24], [1, 96]])
                nc.sync.dma_start_transpose(
                    out=dst, in_=es[0:96, q * 3072:(q + 1) * 3072])
                if dbg_tap == "E" and q == 0:
                    nc.gpsimd.dma_start(out=dbg[s], in_=es[0:96, :])

            def emit_rgb_dma(s):
                nc.gpsimd.dma_start(
                    out=rgbb[s % 2][0:128, :],
                    in_=rgbs[s * 128:(s + 1) * 128, :])

            def emit_bokeh(s):
                if dbg_tap == "eb":
                    nc.gpsimd.dma_start(out=dbg[s], in_=eb[s % 2][0:128, :])
                ostg = ostg_pool.tile([128, 3, QW], F32, name=f"ostg{s}",
                                      tag="ostg")
                with nc.allow_low_precision("fp16 bokeh by design"):
                    for ch in range(4):
                        if ch < 3:
                            # tmpP[(l,dy,dx)] = E * rgb_shift  (tap-minor)
                            dst = bass.AP(
                                tensor=tmpP.tensor, offset=0,
                                ap=[[96 * 81, 128], [81, 96], [9, 9],
                                    [1, 9]])
                            ein = bass.AP(
                                tensor=eb[s % 2].tensor, offset=0,
                                ap=[[EB, 128], [96, 96], [9, 9], [1, 9]])
                            rin = bass.AP(
                                tensor=rgbb[s % 2].tensor,
                                offset=ch * 9 * RGBW,
                                ap=[[RGBF, 128], [1, 96], [RGBW, 9],
                                    [1, 9]])
                            nc.vector.tensor_mul(dst, ein, rin)
                            src_t, tst = tmpP, 81
                        else:
                            src_t, tst = eb[s % 2], 96
                        srcf = 96 * tst

                        def sap(off, cnt):
                            return bass.AP(tensor=src_t.tensor,
                                           offset=off,
                                           ap=[[srcf, 128], [tst, 96],
                                               [1, cnt]])

                        def a40(cnt, t0=0):
                            return bass.AP(tensor=sA40.tensor,
                                           offset=t0,
                                           ap=[[40 * QW, 128],
                                               [40, 96], [1, cnt]])

                        def s20ap(cnt, t0=0):
                            return bass.AP(tensor=s20.tensor, offset=t0,
                                           ap=[[20 * QW, 128], [20, 96],
                                               [1, cnt]])

                        def s10ap(cnt, t0=0):
                            return bass.AP(tensor=s10.tensor, offset=t0,
                                           ap=[[10 * QW, 128], [10, 96],
                                               [1, cnt]])

                        # 80->40 fold, then add tap 80 into column 0
                        nc.vector.tensor_add(a40(40), sap(0, 40),
                                             sap(40, 40))
                        nc.vector.tensor_add(
                            bass.AP(tensor=sA40.tensor, offset=0,
                                    ap=[[40 * QW, 128], [40, 96]]),
                            bass.AP(tensor=sA40.tensor, offset=0,
                                    ap=[[40 * QW, 128], [40, 96]]),
                            bass.AP(tensor=src_t.tensor, offset=80,
                                    ap=[[srcf, 128], [tst, 96]]))
                        nc.vector.tensor_add(s20ap(20), a40(20),
                                             a40(20, t0=20))
                        nc.vector.tensor_add(s10ap(10), s20ap(10),
                                             s20ap(10, t0=10))
                        nc.vector.tensor_reduce(
                            out=uacc[0:128, ch, :], in_=s10ap(10),
                            axis=mybir.AxisListType.X, op=ALU.add)

                    nc.vector.reciprocal(uacc[0:128, 3, :],
                                         uacc[0:128, 3, :])
                    for ch in range(3):
                        nc.vector.tensor_mul(ostg[0:128, ch, :],
                                             uacc[0:128, ch, :],
                                             uacc[0:128, 3, :])

                for ch in range(3):
                    dst = bass.AP(tensor=out.tensor,
                                  offset=ch * R * W + s * SR * W,
                                  ap=[[W, SR], [QW, 4], [1, QW]])
                    src = bass.AP(tensor=ostg.tensor, offset=ch * QW,
                                  ap=[[3 * QW, 128], [1, QW]])
                    nc.gpsimd.dma_start(out=dst, in_=src)

            # ---------------- main row loop ----------------
            emit_rgb_dma(0)
            emit_x36_batch(0)
            kmax = R // 2 + 6
            for k in range(kmax + 1):
                j1 = 2 * k - 1          # conv1 pair (j1, j1+1)
                if j1 <= R + 1:
                    if k % 8 == 0:
                        emit_x36_batch(k + 8)
                    ps1 = psum.tile([128, 384], F32, tag="c1",
                                    name=f"c1_{k}")
                    emit_conv1(k, ps1)
                    nc.scalar.activation(
                        out=h1w[0:128,
                                so1(j1) * RS + 1:so1(j1) * RS + 385],
                        in_=ps1[0:128, 0:384], func=AF.Relu,
                        bias=bias(0, 0, 128))
                    if dbg_tap == "h1":
                        sl = so1(j1) * RS
                        nc.gpsimd.dma_start(
                            out=dbg[:, j1 + 1, :],
                            in_=h1w[0:64, sl:sl + RS])
                        nc.gpsimd.dma_start(
                            out=dbg[:, j1 + 2, :],
                            in_=h1w[64:128, sl:sl + RS])

                w = 2 * k - 8           # conv2 pair (w, w+1), w even
                if 0 <= w <= R:
                    ps2 = psum.tile([128, 384], F32, tag="c2",
                                    name=f"c2_{k}")
                    emit_conv2(w, ps2)
                    emit_conv2_evac(w, ps2)
                    if dbg_tap == "h2":
                        sl = se2(w) * RS
                        nc.gpsimd.dma_start(
                            out=dbg[:, w + 1, :],
                            in_=h2w[0:64, sl:sl + RS])
                        nc.gpsimd.dma_start(
                            out=dbg[:, w + 2, :],
                            in_=h2w[64:128, sl:sl + RS])

                v = 2 * k - 14          # conv3 pair (v, v+1), v even
                if 0 <= v <= R - 2:
                    pse = psum.tile([128, 384], F32, tag="c3e",
                                    name=f"c3e_{k}")
                    pso = psum.tile([128, 384], F32, tag="c3o",
                                    name=f"c3o_{k}")
                    emit_conv3(v, pse, pso)
                    if DIRECT_EXP:
                        emit_exp(v, 0, pse)
                        emit_exp(v, 1, pso)
                    else:
                        er = er_pool.tile([81, 768], F16,
                                          name=f"er_{v}", tag="er")
                        emit_exp_er(v, er, 0, pse)
                        emit_exp_er(v, er, 1, pso)
                        emit_scatter(v, er)
                    if (v + 1) % SR == SR - 1:
                        s = v // SR
                        for q in range(4):
                            emit_xbar_q(s, q)
                        if s + 1 < nstrip:
                            emit_rgb_dma(s + 1)
                        emit_bokeh(s)

    nc.compile()
    return nc


# ------------------------- host side -------------------------

def prep_weights(w1, b1, w2, b2, w3, b3, flip=False):
    if flip:
        perm = np.array([(8 - t // 9) * 9 + t % 9 for t in range(81)])
        w1 = w1[:, :, ::-1, :]
        w2 = w2[:, :, ::-1, :]
        w3 = w3[perm][:, :, ::-1, :]
        b3 = b3[perm]
    wtsb = np.zeros((128, WCOLS), np.float32)
    l1 = w1.transpose(2, 3, 1, 0).reshape(36, 64)
    wtsb[0:36, C_L1:C_L1 + 64] = l1
    wtsb[36:72, C_L1 + 64:C_L1 + 128] = l1
    for kw in range(3):
        kh0 = w2[:, :, 0, kw].T
        kh1 = w2[:, :, 1, kw].T
        kh2 = w2[:, :, 2, kw].T
        a0 = C_A + 128 * kw
        wtsb[0:64, a0:a0 + 64] = kh0
        wtsb[64:128, a0:a0 + 64] = kh1
        wtsb[64:128, a0 + 64:a0 + 128] = kh0
        b0 = C_B + 128 * kw
        wtsb[0:64, b0:b0 + 64] = kh2
        wtsb[0:64, b0 + 64:b0 + 128] = kh1
        wtsb[64:128, b0 + 64:b0 + 128] = kh2
        va = C_LVA + 81 * kw
        wtsb[0:64, va:va + 81] = w3[:, :, 1, kw].T
        wtsb[64:128, va:va + 81] = w3[:, :, 2, kw].T
        vb = C_LVB + 81 * kw
        wtsb[0:64, vb:vb + 81] = w3[:, :, 0, kw].T
        wtsb[64:128, vb:vb + 81] = w3[:, :, 1, kw].T
        vs = C_S + 81 * kw
        wtsb[0:64, vs:vs + 81] = w3[:, :, 2, kw].T
        wtsb[64:128, vs:vs + 81] = w3[:, :, 0, kw].T
    wtb = np.zeros((128, 3), np.float32)
    wtb[0:64, 0] = b1
    wtb[64:128, 0] = b1
    wtb[0:64, 1] = b2
    wtb[64:128, 1] = b2
    wtb[0:81, 2] = b3
    return wtsb.astype(np.float16), wtb


def prep_shard(x, rgb_b, R):
    """x: (4,H,W) fp32 of one (possibly flipped) image; rgb_b: (3,H,W).
    Shard = rows 0..R-1 out; top edge is the image edge (zero pad),
    bottom halo rows come from the rest of the image.

    Returns (x36d, rgbs): odd-pair-packed im2col'd conv1 input and
    per-strip rgb halo blocks with partition p = r' + 32*qx."""
    # conv1 rows -1 .. R+2 (nrows = R+4); taps read x rows -2 .. R+3
    nrows = R + 4
    xp = np.zeros((4, R + 8, RS), np.float32)  # image row i at xp[i+3]
    hi = min(R + 4, H)
    xp[:, 3:3 + hi, 1:385] = x[:, 0:hi, :]
    x36f = np.zeros((36, nrows, RS), np.float16)
    for kh in range(3):
        # conv1 row index i (row r = i-1): tap row r+kh-1 -> xp[i+kh+1]
        sl = xp[:, kh + 1:kh + 1 + nrows, :]
        for kw in range(3):
            blk = np.zeros((4, nrows, RS), np.float32)
            if kw == 0:
                blk[:, :, 1:] = sl[:, :, :-1]
            elif kw == 1:
                blk[:, :, :] = sl
            else:
                blk[:, :, :-1] = sl[:, :, 1:]
            for c in range(4):
                x36f[kh * 12 + kw * 4 + c] = blk[c].astype(np.float16)
    # conv1-out row -1 must be exactly zero (image-edge h1 padding)
    x36f[:, 0, :] = 0
    x36d = np.zeros((72, nrows // 2, RS), np.float16)
    x36d[0:36] = x36f[:, 0::2, :]
    x36d[36:72] = x36f[:, 1::2, :]

    # rgb halo rows -4 .. R+4; partition p = r' + 32*qx
    rgbp = np.zeros((3, R + 8, W + 8), np.float32)
    hi2 = min(R + 4, H)
    rgbp[:, 4:4 + hi2, 4:4 + W] = rgb_b[:, 0:hi2, :]
    nstrip = R // SR
    arr = np.zeros((nstrip * 128, RGBF), np.float16)
    rows = rgbp.astype(np.float16)  # (3, R+8, 392)
    for s in range(nstrip):
        for dy in range(9):
            seg = rows[:, s * SR + dy:s * SR + dy + SR, :]  # (3,SR,392)
            for qx in range(4):
                qseg = seg[:, :, qx * 96:qx * 96 + RGBW]  # (3,SR,104)
                view = arr[s * 128 + qx:s * 128 + 128 + qx:4]
                for c in range(3):
                    view[:, (c * 9 + dy) * RGBW:
                         (c * 9 + dy + 1) * RGBW] = qseg[c]
    return x36d, arr


def _prep_inputs(rgb, depth, w1, b1, w2, b2, w3, b3):
    R = H // 2
    x = np.concatenate([rgb, depth], axis=1)  # (B,4,H,W)
    wt_n = prep_weights(w1, b1, w2, b2, w3, b3, flip=False)
    wt_f = prep_weights(w1, b1, w2, b2, w3, b3, flip=True)
    in_maps = []
    for core in range(NC_):
        bi, half = divmod(core, 2)
        if half == 0:
            xi, ri = x[bi], rgb[bi]
            wtsb, wtb = wt_n
        else:
            xi, ri = x[bi, :, ::-1, :], rgb[bi, :, ::-1, :]
            wtsb, wtb = wt_f
        x36d, rgbs = prep_shard(xi, ri, R)
        in_maps.append({"x36d": x36d, "rgbs": rgbs,
                        "wtsb": wtsb, "wtb": wtb})
    return in_maps


_CACHE = {}


def _get_program(R=H // 2, dbg_tap=None):
    key = (R, dbg_tap)
    if key not in _CACHE:
        _CACHE[key] = build_core_program(R, dbg_tap)
    return _CACHE[key]


def kernel(rgb, depth, w1, b1, w2, b2, w3, b3):
    from concourse.bass_utils import run_bass_kernel_spmd
    rgb = np.asarray(rgb, np.float32)
    depth = np.asarray(depth, np.float32)
    nc = _get_program()
    in_maps = _prep_inputs(rgb, depth, np.asarray(w1, np.float32),
                           np.asarray(b1, np.float32),
                           np.asarray(w2, np.float32),
                           np.asarray(b2, np.float32),
                           np.asarray(w3, np.float32),
                           np.asarray(b3, np.float32))
    res = run_bass_kernel_spmd(nc, in_maps, core_ids=list(range(NC_)),
                               trace=bool(int(os.environ.get("KT_TRACE",
                                                             "0"))))
    R = H // 2
    outp = np.zeros((B, 3, H, W), np.float32)
    for core in range(NC_):
        bi, half = divmod(core, 2)
        o = res.results[core]["out"]
        if half == 0:
            outp[bi, :, 0:R, :] = o
        else:
            outp[bi, :, R:H, :] = o[:, ::-1, :]
    kernel.last_result = res
    return outp


if __name__ == "__main__":
    nc = build_core_program(R=int(sys.argv[1]) if len(sys.argv) > 1 else 32)
    print("built ok")
